# revision 18
# baseline (speedup 1.0000x reference)
"""Trainium2 Bass kernel for EnhancedGNNModel (3-layer GNN message passing).

Strategy (8 NeuronCores, SPMD):
  - Nodes are sharded by dst range: core c owns rows [c*NPC, (c+1)*NPC).
  - Edges are sharded by dst: each core aggregates messages for its own nodes.
  - Per-edge message Linear(concat(h_src, h_dst)) @ W is split algebraically:
        msgs = h[src] @ W_top + h[dst] @ W_bot + b
    so segment_sum(msgs, dst) = (sum_{e->d} h[src]) @ W_top
                                + deg[d] * (h[d] @ W_bot) + deg[d] * b.
    Only S[d] = sum of gathered h[src] rows needs edge-granularity work.
  - Edges are grouped per (super-block of SB dst blocks, src bank); one
    dma_gather call per group pulls h[src] rows into a [128, nch, 128] SBUF
    tile; one bulk is_equal builds all nch one-hot matrices at once; per-chunk
    PE matmuls accumulate S^T per dst block in PSUM.
  - h lives transposed ([HID, node]) in SBUF for all dense matmuls; dense
    weights run in bf16. Updated rows are transposed back via the PE, written
    to DRAM, and AllGathered so every core has the full h for the next
    layer's gathers.
"""
import hashlib
import ml_dtypes
import numpy as np
from contextlib import ExitStack
from dataclasses import dataclass

import concourse.bass as bass
import concourse.tile as tile
from concourse import bacc, mybir
from concourse.bass_utils import run_bass_kernel_spmd

F32 = mybir.dt.float32
BF16 = mybir.dt.bfloat16
I16 = mybir.dt.int16
BF = ml_dtypes.bfloat16


@dataclass(frozen=True)
class Cfg:
    n_nodes: int = 50000
    n_edges: int = 800000
    feat: int = 64
    hid: int = 128
    layers: int = 3
    n_cores: int = 8
    sb: int = 5  # dst blocks per super-block (one gather group per bank)
    gcap: int = 1024  # max indices per dma_gather call (ring-capacity limit)
    dma_scratch: int = 16384
    n_queues: int = 1

    @property
    def npc(self):  # nodes per core (multiple of 128)
        per = -(-self.n_nodes // self.n_cores)
        return -(-per // 128) * 128

    @property
    def n_pad(self):
        return self.npc * self.n_cores

    @property
    def bank_rows(self):
        b = self.n_pad // 2
        assert b <= 32768, "dma_gather int16 index limit"
        return b

    @property
    def n_blocks(self):
        return self.npc // 128


def plan_edges(cfg: Cfg, edge_indices: np.ndarray):
    """Host-side sharding plan. Edge order per core:
    super-block s -> bank k -> block b -> edges (padded to 128 per (b,k)).
    Returns (meta, per_core). meta['calls'] is a list of
    (nch, off16, chunk0, bank) and meta['chunk_block'] maps chunk -> block.
    """
    src = np.asarray(edge_indices[0], dtype=np.int64)
    dst = np.asarray(edge_indices[1], dtype=np.int64)
    C, npc, nb, br = cfg.n_cores, cfg.npc, cfg.n_blocks, cfg.bank_rows

    core = dst // npc
    ldst = dst - core * npc
    block = ldst >> 7
    dst_rel = ldst & 127
    bank = (src >= br).astype(np.int64)
    bidx = src - bank * br

    counts = np.zeros((C, nb, 2), dtype=np.int64)
    np.add.at(counts, (core, block, bank), 1)
    caps = counts.max(axis=0)  # [nb, 2]
    caps = np.where(caps > 0, ((caps + 127) // 128) * 128, 0).astype(np.int64)

    key = (core * nb + block) * 2 + bank
    order = np.argsort(key, kind="stable")
    bidx_s, rel_s = bidx[order], dst_rel[order]
    group_sizes = np.bincount(key[order], minlength=C * nb * 2)
    group_starts = np.concatenate([[0], np.cumsum(group_sizes)])

    # super-block partition of blocks
    sblocks = [list(range(s, min(s + cfg.sb, nb)))
               for s in range(0, nb, cfg.sb)]

    # static layout: order of (s, k, b) groups; each group caps[b,k] slots
    layout = []  # (b, k, cap)
    for bs in sblocks:
        for k in range(2):
            for b in bs:
                if caps[b, k]:
                    layout.append((b, k, int(caps[b, k])))
    total_cap = sum(g[2] for g in layout)
    n_chunks = total_cap // 128

    # chunk -> block map and call list: one call per (block, bank) group
    # (split at gcap if oversized), so per-core padding is always a call
    # TAIL and can be skipped via idx=-1 + per-core num_idxs_reg.
    chunk_block = []
    calls = []  # (nch, off16, chunk0, bank, b, q0)
    off = 0
    for bs in sblocks:
        for k in range(2):
            for g in [g for g in layout if g[0] in bs and g[1] == k]:
                b, _, cap = g
                chunk_block += [b] * (cap // 128)
                q = 0
                while q < cap:
                    cc = min(cfg.gcap, cap - q)
                    calls.append((cc // 128, (off + q) // 16, (off + q) // 128,
                                  k, b, q))
                    q += cc
                off += cap
    assert off == total_cap and len(chunk_block) == n_chunks

    per_core_arr = []
    for c in range(C):
        idx_flat = np.full(total_cap, -1, dtype=np.int16)
        rel_flat = np.full(total_cap, -1.0, dtype=np.float32)
        pos = 0
        for (b, k, cap) in layout:
            g = (c * nb + b) * 2 + k
            s0, n = group_starts[g], group_sizes[g]
            idx_flat[pos:pos + n] = bidx_s[s0:s0 + n].astype(np.int16)
            rel_flat[pos:pos + n] = rel_s[s0:s0 + n].astype(np.float32)
            pos += cap
        assert pos == total_cap

        # per-core valid-index count per call (trailing -1 padding skipped);
        # a call window with no real indices gets one idx-0 slot
        cnts = []
        for (nch, off16, _, k, b, q0) in calls:
            g = (c * nb + b) * 2 + k
            n = int(group_sizes[g]) - q0
            if n < 1:
                idx_flat[off16 * 16] = 0
                n = 1
            cnts.append(min(n, nch * 128))
        cnts = np.array(cnts, dtype=np.int32)[None, :]

        # wrap indices per call: idx j of a call sits at [j % 16, j // 16]
        idx_w = np.zeros((16, total_cap // 16), dtype=np.int16)
        for (nch, off16, _, _, _, _) in calls:
            cc = nch * 128
            seg = idx_flat[off16 * 16: off16 * 16 + cc].reshape(cc // 16, 16).T
            idx_w[:, off16: off16 + cc // 16] = seg
        idx_w = np.tile(idx_w, (8, 1))
        rel_t = rel_flat.reshape(n_chunks, 128).T.astype(BF)  # [128, n_chunks]

        deg = np.zeros(npc, dtype=np.float32)
        m = core == c
        np.add.at(deg, ldst[m], 1.0)
        mask = np.zeros(npc, dtype=np.float32)
        lo = c * npc
        mask[: max(0, min(npc, cfg.n_nodes - lo))] = 1.0
        per_core_arr.append(dict(idxs=idx_w, dstrel=rel_t, call_cnts=cnts,
                                 mask=mask[None, :].astype(BF),
                                 deg_bcast=np.tile(deg[None, :], (128, 1)).astype(BF)))

    meta = dict(caps=caps, calls=calls, chunk_block=chunk_block,
                sblocks=sblocks, n_chunks=n_chunks, total_cap=total_cap)
    return meta, per_core_arr


def build_program(cfg: Cfg, meta):
    C, npc, nb = cfg.n_cores, cfg.npc, cfg.n_blocks
    FEAT, HID, L = cfg.feat, cfg.hid, cfg.layers
    n_chunks, total_cap = meta["n_chunks"], meta["total_cap"]
    calls, chunk_block, sblocks = meta["calls"], meta["chunk_block"], meta["sblocks"]
    max_nch = max(c[0] for c in calls)

    # first/last chunk index per block (chunks of one block are contiguous
    # except for the bank split; find global first/last over all its chunks)
    first_chunk = {}
    last_chunk = {}
    for t, b in enumerate(chunk_block):
        first_chunk.setdefault(b, t)
        last_chunk[b] = t

    nc = bacc.Bacc("TRN2", target_bir_lowering=False, debug=False, num_devices=C,
                   dynamic_dma_scratch_size=cfg.dma_scratch,
                   num_swdge_queues=cfg.n_queues)

    def inp(name, shape, dt=F32):
        return nc.dram_tensor(name, shape, dt, kind="ExternalInput").ap()

    at_d = inp("at_own", [FEAT, npc])
    embW_d = inp("emb_W", [FEAT, HID])
    embb_d = inp("emb_b", [1, HID])
    mtop_d = inp("msg_top", [L * HID, HID], BF16)
    mbot_d = inp("msg_bot", [L * HID, HID], BF16)
    mb_d = inp("msg_bias", [L, HID], BF16)
    utop_d = inp("upd_top", [L * HID, HID], BF16)
    ubot_d = inp("upd_bot", [L * HID, HID], BF16)
    ub_d = inp("upd_bias", [L, HID], BF16)
    ow1_d = inp("out_W1", [HID, HID // 2])
    ob1_d = inp("out_b1", [HID // 2, 1])
    ow2_d = inp("out_W2", [HID // 2, HID // 4])
    ob2_d = inp("out_b2", [HID // 4, 1])
    ow3_d = inp("out_W3", [HID // 4, 1])
    ob3_d = inp("out_b3", [1, 1])
    mask_d = inp("mask", [1, npc], BF16)
    degb_d = inp("deg_bcast", [128, npc], BF16)
    idx_d = inp("idxs", [128, total_cap // 16], I16)
    rel_d = inp("dstrel", [128, n_chunks], BF16)
    cnt_d = inp("call_cnts", [1, len(calls)], mybir.dt.int32)
    out_d = nc.dram_tensor("result", [1, 1], F32, kind="ExternalOutput").ap()

    with tile.TileContext(nc) as tc, ExitStack() as ctx:
        const = ctx.enter_context(tc.tile_pool(name="const", bufs=1))
        gpool = ctx.enter_context(tc.tile_pool(name="g", bufs=2))
        ohpool = ctx.enter_context(tc.tile_pool(name="oh", bufs=2))
        spool = ctx.enter_context(tc.tile_pool(name="s", bufs=8))
        rpool = ctx.enter_context(tc.tile_pool(name="rows", bufs=4))
        ps_s = ctx.enter_context(tc.tile_pool(name="ps_s", bufs=6, space="PSUM"))
        ps_d = ctx.enter_context(tc.tile_pool(name="ps_d", bufs=2, space="PSUM"))
        dram = ctx.enter_context(tc.tile_pool(name="dram", bufs=2, space="DRAM"))

        def load_const(name, ap_dram, shape, dt=F32):
            t = const.tile(shape, dt, name=name, tag=name)
            nc.sync.dma_start(out=t[:], in_=ap_dram)
            return t

        embW_sb = load_const("embW_sb", embW_d[:], [FEAT, HID])
        embb_sb = load_const("embb_sb", embb_d[:], [1, HID])
        mtop_sb = [load_const(f"mtop{l}", mtop_d[l * HID:(l + 1) * HID, :], [HID, HID], BF16) for l in range(L)]
        mbot_sb = [load_const(f"mbot{l}", mbot_d[l * HID:(l + 1) * HID, :], [HID, HID], BF16) for l in range(L)]
        mb_sb = [load_const(f"mb{l}", mb_d[l:l + 1, :], [1, HID], BF16) for l in range(L)]
        utop_sb = [load_const(f"utop{l}", utop_d[l * HID:(l + 1) * HID, :], [HID, HID], BF16) for l in range(L)]
        ubot_sb = [load_const(f"ubot{l}", ubot_d[l * HID:(l + 1) * HID, :], [HID, HID], BF16) for l in range(L)]
        ub_sb = [load_const(f"ub{l}", ub_d[l:l + 1, :], [1, HID], BF16) for l in range(L)]
        ow1_sb = load_const("ow1_sb", ow1_d[:], [HID, HID // 2])
        ob1_sb = load_const("ob1_sb", ob1_d[:], [HID // 2, 1])
        ow2_sb = load_const("ow2_sb", ow2_d[:], [HID // 2, HID // 4])
        ob2_sb = load_const("ob2_sb", ob2_d[:], [HID // 4, 1])
        ow3_sb = load_const("ow3_sb", ow3_d[:], [HID // 4, 1])
        ob3_sb = load_const("ob3_sb", ob3_d[:], [1, 1])
        mask_sb = load_const("mask_sb", mask_d[:], [1, npc], BF16)
        degb_sb = load_const("degb_sb", degb_d[:], [128, npc], BF16)
        deg_sb = degb_sb[0:1, :]
        idx_sb = load_const("idx_sb", idx_d[:], [128, total_cap // 16], I16)
        rel_sb = load_const("rel_sb", rel_d[:], [128, n_chunks], BF16)
        cnt_sb = load_const("cnt_sb", cnt_d[:], [1, len(calls)], mybir.dt.int32)

        iotah_sb = const.tile([128, 128], BF16)
        nc.gpsimd.iota(iotah_sb[:], [[1, 128]], channel_multiplier=0,
                       allow_small_or_imprecise_dtypes=True)
        iota_sb = const.tile([128, 128], F32)
        nc.gpsimd.iota(iota_sb[:], [[1, 128]], channel_multiplier=0,
                       allow_small_or_imprecise_dtypes=True)
        iota_col = const.tile([128, 1], F32)
        nc.gpsimd.iota(iota_col[:], [[1, 1]], channel_multiplier=1,
                       allow_small_or_imprecise_dtypes=True)
        ident_sb = const.tile([128, 128], F32)
        nc.vector.tensor_scalar(ident_sb[:], iota_sb[:], iota_col[:], None,
                                op0=mybir.AluOpType.is_equal)

        hT = const.tile([128, npc], F32)  # h transposed, own nodes

        def emit_h_rows():
            """Transpose hT to row-major, DMA to DRAM, AllGather full h."""
            rows_dram = dram.tile([npc, HID], BF16, tag="rows_d")
            for b in range(nb):
                blk = slice(b * 128, (b + 1) * 128)
                pt = ps_d.tile([128, 512], F32, tag="d")
                nc.tensor.transpose(pt[:, :128], hT[:, blk], ident_sb[:])
                r_sb = rpool.tile([128, 128], BF16, tag="r")
                nc.scalar.copy(r_sb[:], pt[:, :128])
                nc.sync.dma_start(out=rows_dram[blk, :], in_=r_sb[:])
            hfull = dram.tile([cfg.n_pad, HID], BF16, tag="hfull_d", addr_space="Shared")
            nc.gpsimd.collective_compute(
                "AllGather", mybir.AluOpType.bypass,
                replica_groups=[list(range(C))],
                ins=[rows_dram.opt()], outs=[hfull.opt()],
            )
            return hfull

        # ---- embedding: hT = emb_W^T @ at_own + emb_b (mask-broadcast) ----
        with tc.tile_pool(name="atp", bufs=1) as atp:
            at_sb = atp.tile([FEAT, npc], F32, name="at_sb")
            nc.sync.dma_start(out=at_sb[:], in_=at_d[:])
            maskf_sb = atp.tile([1, npc], F32, name="maskf_sb")
            nc.vector.tensor_copy(maskf_sb[:], mask_sb[:])
            for j0 in range(0, npc, 512):
                jn = min(512, npc - j0)
                pe = ps_d.tile([128, 512], F32, tag="d", name="pe")
                nc.tensor.matmul(pe[:, :jn], embW_sb[:, :], at_sb[:, j0:j0 + jn],
                                 start=True, stop=False)
                nc.tensor.matmul(pe[:, :jn], embb_sb[:, :], maskf_sb[:, j0:j0 + jn],
                                 start=False, stop=True)
                nc.scalar.copy(hT[:, j0:j0 + jn], pe[:, :jn])
        hfull = emit_h_rows()

        # warm the gather buffers so skipped (-1) tail slots hold finite
        # values (garbage * zero one-hot must not produce NaN)
        for _ in range(2):
            gw = gpool.tile([128, cfg.gcap // 128, 128], BF16, tag="g", name="gw")
            nc.vector.memset(gw[:, :, :], 0.0)

        cnt_reg = nc.gpsimd.alloc_register("gather_cnt")

        # ---- message passing layers ----
        for l in range(L):
            psums = {}
            for ci, (nch, off16, chunk0, k, _, _) in enumerate(calls):
                cc = nch * 128
                g = gpool.tile([128, cfg.gcap // 128, 128], BF16, tag="g")
                nc.gpsimd.reg_load(cnt_reg, cnt_sb[0:1, ci:ci + 1])
                nc.gpsimd.dma_gather(
                    g[:, :nch, :],
                    hfull[k * cfg.bank_rows:(k + 1) * cfg.bank_rows, :],
                    idx_sb[:, off16: off16 + cc // 16],
                    num_idxs=cc, num_idxs_reg=cnt_reg, elem_size=HID,
                    queue_num=ci % cfg.n_queues,
                )
                oh = ohpool.tile([128, cfg.gcap // 128, 128], BF16, tag="oh")
                nc.vector.tensor_tensor(
                    oh[:, :nch, :],
                    iotah_sb[:, :].unsqueeze(1).broadcast_to([128, nch, 128]),
                    rel_sb[:, chunk0:chunk0 + nch].unsqueeze(2).broadcast_to([128, nch, 128]),
                    op=mybir.AluOpType.is_equal)
                for t in range(nch):
                    b = chunk_block[chunk0 + t]
                    if first_chunk[b] == chunk0 + t:
                        psums[b] = ps_s.tile([128, 128], F32, tag="ps_s",
                                             name=f"psum_{b % 16}")
                    nc.tensor.matmul(psums[b][:], g[:, t, :], oh[:, t, :],
                                     start=(first_chunk[b] == chunk0 + t),
                                     stop=(last_chunk[b] == chunk0 + t))
                # when this call closes a super-block's bank-1 run, flush its
                # blocks' dense ops
                next_chunk0 = (calls[ci + 1][2] if ci + 1 < len(calls)
                               else n_chunks)
                done_blocks = [b for b in list(psums)
                               if last_chunk[b] < next_chunk0]
                for b in sorted(done_blocks):
                    blk = slice(b * 128, (b + 1) * 128)
                    s_sb = spool.tile([128, 128], BF16, tag="s")
                    nc.scalar.copy(s_sb[:], psums.pop(b)[:])
                    hTb_sb = spool.tile([128, 128], BF16, tag="s", name="hTb_sb")
                    nc.scalar.copy(hTb_sb[:], hT[:, blk])
                    hdeg_sb = spool.tile([128, 128], BF16, tag="s", name="hdeg_sb")
                    nc.vector.tensor_mul(hdeg_sb[:], hTb_sb[:], degb_sb[:, blk])
                    pa = ps_d.tile([128, 512], F32, tag="d")
                    nc.tensor.matmul(pa[:, :128], mtop_sb[l][:], s_sb[:], start=True, stop=False)
                    nc.tensor.matmul(pa[:, :128], mbot_sb[l][:], hdeg_sb[:], start=False, stop=False)
                    nc.tensor.matmul(pa[:, :128], mb_sb[l][:], deg_sb[:, blk], start=False, stop=True)
                    agg_sb = spool.tile([128, 128], BF16, tag="s")
                    nc.scalar.copy(agg_sb[:], pa[:, :128])

                    pu = ps_d.tile([128, 512], F32, tag="d")
                    nc.tensor.matmul(pu[:, :128], utop_sb[l][:], hTb_sb[:], start=True, stop=False)
                    nc.tensor.matmul(pu[:, :128], ubot_sb[l][:], agg_sb[:], start=False, stop=False)
                    nc.tensor.matmul(pu[:, :128], ub_sb[l][:], mask_sb[:, blk], start=False, stop=True)
                    relu_sb = spool.tile([128, 128], F32, tag="s")
                    nc.scalar.activation(relu_sb[:], pu[:, :128],
                                         mybir.ActivationFunctionType.Relu)
                    nc.vector.tensor_add(hT[:, blk], relu_sb[:], hT[:, blk])
            assert not psums
            # blocks with zero chunks (no in-edges anywhere): still need dense
            for b in range(nb):
                if b in first_chunk:
                    continue
                blk = slice(b * 128, (b + 1) * 128)
                hTb_sb = spool.tile([128, 128], BF16, tag="s", name="hTb_sb")
                nc.scalar.copy(hTb_sb[:], hT[:, blk])
                pu = ps_d.tile([128, 512], F32, tag="d")
                nc.tensor.matmul(pu[:, :128], utop_sb[l][:], hTb_sb[:], start=True, stop=False)
                nc.tensor.matmul(pu[:, :128], ub_sb[l][:], mask_sb[:, blk], start=False, stop=True)
                relu_sb = spool.tile([128, 128], F32, tag="s")
                nc.scalar.activation(relu_sb[:], pu[:, :128],
                                     mybir.ActivationFunctionType.Relu)
                nc.vector.tensor_add(hT[:, blk], relu_sb[:], hT[:, blk])
            if l < L - 1:
                hfull = emit_h_rows()

        # ---- readout: g = mean(h) ; out = MLP(g) ----
        part_sb = spool.tile([128, 1], F32, tag="s")
        nc.vector.tensor_reduce(part_sb[:], hT[:, :], axis=mybir.AxisListType.X,
                                op=mybir.AluOpType.add)
        part_dram = dram.tile([128, 1], F32, tag="pt_d")
        gsum_dram = dram.tile([128, 1], F32, tag="gs_d", addr_space="Shared")
        nc.sync.dma_start(out=part_dram[:], in_=part_sb[:])
        nc.gpsimd.collective_compute(
            "AllReduce", mybir.AluOpType.add,
            replica_groups=[list(range(C))],
            ins=[part_dram.opt()], outs=[gsum_dram.opt()],
        )
        gsum_sb = spool.tile([128, 1], F32, tag="s")
        nc.sync.dma_start(out=gsum_sb[:], in_=gsum_dram[:])

        p1 = ps_d.tile([128, 512], F32, tag="d")
        nc.tensor.matmul(p1[:HID // 2, :1], ow1_sb[:], gsum_sb[:], start=True, stop=True)
        o1_sb = spool.tile([HID // 2, 1], F32, tag="o1")
        nc.scalar.activation(o1_sb[:], p1[:HID // 2, :1],
                             mybir.ActivationFunctionType.Relu,
                             bias=ob1_sb[:], scale=1.0 / cfg.n_nodes)
        p2 = ps_d.tile([128, 512], F32, tag="d")
        nc.tensor.matmul(p2[:HID // 4, :1], ow2_sb[:], o1_sb[:], start=True, stop=True)
        o2_sb = spool.tile([HID // 4, 1], F32, tag="o2")
        nc.scalar.activation(o2_sb[:], p2[:HID // 4, :1],
                             mybir.ActivationFunctionType.Relu, bias=ob2_sb[:])
        p3 = ps_d.tile([128, 512], F32, tag="d")
        nc.tensor.matmul(p3[:1, :1], ow3_sb[:], o2_sb[:], start=True, stop=True)
        o3_sb = spool.tile([1, 1], F32, tag="o3")
        nc.scalar.activation(o3_sb[:], p3[:1, :1],
                             mybir.ActivationFunctionType.Identity, bias=ob3_sb[:])
        nc.sync.dma_start(out=out_d[:], in_=o3_sb[:])

    nc.compile()
    return nc


def make_in_maps(cfg: Cfg, inputs, per_core_arr):
    C, npc, FEAT, HID, L = cfg.n_cores, cfg.npc, cfg.feat, cfg.hid, cfg.layers
    af = np.asarray(inputs["atom_features"], np.float32)
    at_pad = np.zeros((cfg.n_pad, FEAT), np.float32)
    at_pad[:cfg.n_nodes] = af
    msg_W = np.asarray(inputs["msg_W"], np.float32)
    upd_W = np.asarray(inputs["upd_W"], np.float32)
    shared = dict(
        emb_W=np.asarray(inputs["emb_W"], np.float32),
        emb_b=np.asarray(inputs["emb_b"], np.float32)[None, :],
        msg_top=msg_W[:, :HID, :].reshape(L * HID, HID).astype(BF),
        msg_bot=msg_W[:, HID:, :].reshape(L * HID, HID).astype(BF),
        msg_bias=np.asarray(inputs["msg_b"], np.float32).astype(BF),
        upd_top=upd_W[:, :HID, :].reshape(L * HID, HID).astype(BF),
        upd_bot=upd_W[:, HID:, :].reshape(L * HID, HID).astype(BF),
        upd_bias=np.asarray(inputs["upd_b"], np.float32).astype(BF),
        out_W1=np.asarray(inputs["out_W1"], np.float32),
        out_b1=np.asarray(inputs["out_b1"], np.float32)[:, None],
        out_W2=np.asarray(inputs["out_W2"], np.float32),
        out_b2=np.asarray(inputs["out_b2"], np.float32)[:, None],
        out_W3=np.asarray(inputs["out_W3"], np.float32),
        out_b3=np.asarray(inputs["out_b3"], np.float32)[:, None],
    )
    in_maps = []
    for c in range(C):
        d = dict(shared)
        d["at_own"] = at_pad[c * npc:(c + 1) * npc].T.copy()
        d.update(per_core_arr[c])
        in_maps.append(d)
    return in_maps


_prog_cache = {}


def run(cfg: Cfg, inputs, trace=False):
    meta, per_core_arr = plan_edges(cfg, np.asarray(inputs["edge_indices"]))
    key = (cfg, hashlib.sha1(meta["caps"].tobytes()).hexdigest())
    if key not in _prog_cache:
        _prog_cache[key] = build_program(cfg, meta)
    nc = _prog_cache[key]
    in_maps = make_in_maps(cfg, inputs, per_core_arr)
    res = run_bass_kernel_spmd(nc, in_maps, core_ids=list(range(cfg.n_cores)),
                               trace=trace)
    out = res.results[0]["result"].astype(np.float32)
    return out, res


def kernel(**inputs) -> np.ndarray:
    out, _ = run(Cfg(), inputs)
    return out


# revision 29
# speedup vs baseline: 1.3155x; 1.3155x over previous
"""Trainium2 Bass kernel for EnhancedGNNModel (3-layer GNN message passing).

Strategy (8 NeuronCores, SPMD):
  - Nodes are sharded by dst range: core c owns rows [c*NPC, (c+1)*NPC).
  - Edges are sharded by dst: each core aggregates messages for its own nodes.
  - Per-edge message Linear(concat(h_src, h_dst)) @ W is split algebraically:
        msgs = h[src] @ W_top + h[dst] @ W_bot + b
    so segment_sum(msgs, dst) = (sum_{e->d} h[src]) @ W_top
                                + deg[d] * (h[d] @ W_bot) + deg[d] * b.
    Only S[d] = sum of gathered h[src] rows needs edge-granularity work.
  - Edges are grouped per (super-block of SB dst blocks, src bank); one
    dma_gather call per group pulls h[src] rows into a [128, nch, 128] SBUF
    tile; one bulk is_equal builds all nch one-hot matrices at once; per-chunk
    PE matmuls accumulate S^T per dst block in PSUM.
  - h lives transposed ([HID, node]) in SBUF for all dense matmuls; dense
    weights run in bf16. Updated rows are transposed back via the PE, written
    to DRAM, and AllGathered so every core has the full h for the next
    layer's gathers.
"""
import hashlib
import ml_dtypes
import numpy as np
from contextlib import ExitStack
from dataclasses import dataclass

import concourse.bass as bass
import concourse.tile as tile
from concourse import bacc, mybir
from concourse.bass_utils import run_bass_kernel_spmd

F32 = mybir.dt.float32
BF16 = mybir.dt.bfloat16
I16 = mybir.dt.int16
BF = ml_dtypes.bfloat16


@dataclass(frozen=True)
class Cfg:
    n_nodes: int = 50000
    n_edges: int = 800000
    feat: int = 64
    hid: int = 128
    layers: int = 3
    n_cores: int = 8
    sb: int = 5  # dst blocks per super-block (one gather group per bank)
    gcap: int = 1024  # max indices per dma_gather call (ring-capacity limit)
    dma_scratch: int = 16384
    n_queues: int = 1

    @property
    def npc(self):  # nodes per core (multiple of 128)
        per = -(-self.n_nodes // self.n_cores)
        return -(-per // 128) * 128

    @property
    def n_pad(self):
        return self.npc * self.n_cores

    @property
    def bank_rows(self):
        b = self.n_pad // 2
        assert b <= 32768, "dma_gather int16 index limit"
        return b

    @property
    def n_blocks(self):
        return self.npc // 128


def plan_edges(cfg: Cfg, edge_indices: np.ndarray):
    """Host-side sharding plan. Edge order per core:
    super-block s -> bank k -> block b -> edges (padded to 128 per (b,k)).
    Returns (meta, per_core). meta['calls'] is a list of
    (nch, off16, chunk0, bank) and meta['chunk_block'] maps chunk -> block.
    """
    src = np.asarray(edge_indices[0], dtype=np.int64)
    dst = np.asarray(edge_indices[1], dtype=np.int64)
    C, npc, nb, br = cfg.n_cores, cfg.npc, cfg.n_blocks, cfg.bank_rows

    core = dst // npc
    ldst = dst - core * npc
    block = ldst >> 7
    dst_rel = ldst & 127
    bank = (src >= br).astype(np.int64)
    bidx = src - bank * br

    counts = np.zeros((C, nb, 2), dtype=np.int64)
    np.add.at(counts, (core, block, bank), 1)
    caps = counts.max(axis=0)  # [nb, 2]
    caps = np.where(caps > 0, ((caps + 127) // 128) * 128, 0).astype(np.int64)

    key = (core * nb + block) * 2 + bank
    order = np.argsort(key, kind="stable")
    bidx_s, rel_s = bidx[order], dst_rel[order]
    group_sizes = np.bincount(key[order], minlength=C * nb * 2)
    group_starts = np.concatenate([[0], np.cumsum(group_sizes)])

    # super-block partition of blocks
    sblocks = [list(range(s, min(s + cfg.sb, nb)))
               for s in range(0, nb, cfg.sb)]

    # static layout: order of (s, k, b) groups; each group caps[b,k] slots
    layout = []  # (b, k, cap)
    for bs in sblocks:
        for k in range(2):
            for b in bs:
                if caps[b, k]:
                    layout.append((b, k, int(caps[b, k])))
    total_cap = sum(g[2] for g in layout)
    n_chunks = total_cap // 128

    # chunk -> block map and call list (calls split at gcap, never across
    # (s, k) boundaries; all chunks of one call share the bank)
    chunk_block = []
    calls = []  # (nch, off16, chunk0, bank)
    off = 0
    for bs in sblocks:
        for k in range(2):
            groups = [g for g in layout if g[0] in bs and g[1] == k]
            run = sum(g[2] for g in groups)
            if run == 0:
                continue
            for g in groups:
                chunk_block += [g[0]] * (g[2] // 128)
            q = 0
            while q < run:
                cc = min(cfg.gcap, run - q)
                calls.append((cc // 128, (off + q) // 16, (off + q) // 128, k))
                q += cc
            off += run
    assert off == total_cap and len(chunk_block) == n_chunks

    per_core_arr = []
    for c in range(C):
        idx_flat = np.zeros(total_cap, dtype=np.int16)
        rel_flat = np.full(total_cap, -1.0, dtype=np.float32)
        pos = 0
        for (b, k, cap) in layout:
            g = (c * nb + b) * 2 + k
            s0, n = group_starts[g], group_sizes[g]
            idx_flat[pos:pos + n] = bidx_s[s0:s0 + n].astype(np.int16)
            rel_flat[pos:pos + n] = rel_s[s0:s0 + n].astype(np.float32)
            pos += cap
        assert pos == total_cap

        # wrap indices per call: idx j of a call sits at [j % 16, j // 16]
        idx_w = np.zeros((16, total_cap // 16), dtype=np.int16)
        for (nch, off16, _, _) in calls:
            cc = nch * 128
            seg = idx_flat[off16 * 16: off16 * 16 + cc].reshape(cc // 16, 16).T
            idx_w[:, off16: off16 + cc // 16] = seg
        idx_w = np.tile(idx_w, (8, 1))
        rel_t = rel_flat.reshape(n_chunks, 128).T.astype(BF)  # [128, n_chunks]

        deg = np.zeros(npc, dtype=np.float32)
        m = core == c
        np.add.at(deg, ldst[m], 1.0)
        mask = np.zeros(npc, dtype=np.float32)
        lo = c * npc
        mask[: max(0, min(npc, cfg.n_nodes - lo))] = 1.0
        per_core_arr.append(dict(idxs=idx_w, dstrel=rel_t,
                                 mask=mask[None, :].astype(BF),
                                 deg_bcast=np.tile(deg[None, :], (128, 1)).astype(BF)))

    meta = dict(caps=caps, calls=calls, chunk_block=chunk_block,
                sblocks=sblocks, n_chunks=n_chunks, total_cap=total_cap)
    return meta, per_core_arr


def build_program(cfg: Cfg, meta):
    C, npc, nb = cfg.n_cores, cfg.npc, cfg.n_blocks
    FEAT, HID, L = cfg.feat, cfg.hid, cfg.layers
    n_chunks, total_cap = meta["n_chunks"], meta["total_cap"]
    calls, chunk_block, sblocks = meta["calls"], meta["chunk_block"], meta["sblocks"]
    max_nch = max(c[0] for c in calls)

    # first/last chunk index per block (chunks of one block are contiguous
    # except for the bank split; find global first/last over all its chunks)
    first_chunk = {}
    last_chunk = {}
    for t, b in enumerate(chunk_block):
        first_chunk.setdefault(b, t)
        last_chunk[b] = t

    nc = bacc.Bacc("TRN2", target_bir_lowering=False, debug=False, num_devices=C,
                   dynamic_dma_scratch_size=cfg.dma_scratch,
                   num_swdge_queues=cfg.n_queues)

    def inp(name, shape, dt=F32):
        return nc.dram_tensor(name, shape, dt, kind="ExternalInput").ap()

    at_d = inp("at_own", [FEAT, npc])
    embW_d = inp("emb_W", [FEAT, HID])
    embb_d = inp("emb_b", [1, HID])
    mtop_d = inp("msg_top", [L * HID, HID], BF16)
    mbot_d = inp("msg_bot", [L * HID, HID], BF16)
    mb_d = inp("msg_bias", [L, HID], BF16)
    utop_d = inp("upd_top", [L * HID, HID], BF16)
    ubot_d = inp("upd_bot", [L * HID, HID], BF16)
    ub_d = inp("upd_bias", [L, HID], BF16)
    ow1_d = inp("out_W1", [HID, HID // 2])
    ob1_d = inp("out_b1", [HID // 2, 1])
    ow2_d = inp("out_W2", [HID // 2, HID // 4])
    ob2_d = inp("out_b2", [HID // 4, 1])
    ow3_d = inp("out_W3", [HID // 4, 1])
    ob3_d = inp("out_b3", [1, 1])
    mask_d = inp("mask", [1, npc], BF16)
    degb_d = inp("deg_bcast", [128, npc], BF16)
    idx_d = inp("idxs", [128, total_cap // 16], I16)
    rel_d = inp("dstrel", [128, n_chunks], BF16)
    out_d = nc.dram_tensor("result", [1, 1], F32, kind="ExternalOutput").ap()

    with tile.TileContext(nc) as tc, ExitStack() as ctx:
        const = ctx.enter_context(tc.tile_pool(name="const", bufs=1))
        gpool = ctx.enter_context(tc.tile_pool(name="g", bufs=2))
        ohpool = ctx.enter_context(tc.tile_pool(name="oh", bufs=2))
        spool = ctx.enter_context(tc.tile_pool(name="s", bufs=8))
        rpool = ctx.enter_context(tc.tile_pool(name="rows", bufs=4))
        ps_s = ctx.enter_context(tc.tile_pool(name="ps_s", bufs=6, space="PSUM"))
        ps_d = ctx.enter_context(tc.tile_pool(name="ps_d", bufs=2, space="PSUM"))
        dram = ctx.enter_context(tc.tile_pool(name="dram", bufs=2, space="DRAM"))

        def load_const(name, ap_dram, shape, dt=F32):
            t = const.tile(shape, dt, name=name, tag=name)
            nc.sync.dma_start(out=t[:], in_=ap_dram)
            return t

        embW_sb = load_const("embW_sb", embW_d[:], [FEAT, HID])
        embb_sb = load_const("embb_sb", embb_d[:], [1, HID])
        mtop_sb = [load_const(f"mtop{l}", mtop_d[l * HID:(l + 1) * HID, :], [HID, HID], BF16) for l in range(L)]
        mbot_sb = [load_const(f"mbot{l}", mbot_d[l * HID:(l + 1) * HID, :], [HID, HID], BF16) for l in range(L)]
        mb_sb = [load_const(f"mb{l}", mb_d[l:l + 1, :], [1, HID], BF16) for l in range(L)]
        utop_sb = [load_const(f"utop{l}", utop_d[l * HID:(l + 1) * HID, :], [HID, HID], BF16) for l in range(L)]
        ubot_sb = [load_const(f"ubot{l}", ubot_d[l * HID:(l + 1) * HID, :], [HID, HID], BF16) for l in range(L)]
        ub_sb = [load_const(f"ub{l}", ub_d[l:l + 1, :], [1, HID], BF16) for l in range(L)]
        ow1_sb = load_const("ow1_sb", ow1_d[:], [HID, HID // 2])
        ob1_sb = load_const("ob1_sb", ob1_d[:], [HID // 2, 1])
        ow2_sb = load_const("ow2_sb", ow2_d[:], [HID // 2, HID // 4])
        ob2_sb = load_const("ob2_sb", ob2_d[:], [HID // 4, 1])
        ow3_sb = load_const("ow3_sb", ow3_d[:], [HID // 4, 1])
        ob3_sb = load_const("ob3_sb", ob3_d[:], [1, 1])
        mask_sb = load_const("mask_sb", mask_d[:], [1, npc], BF16)
        degb_sb = load_const("degb_sb", degb_d[:], [128, npc], BF16)
        deg_sb = degb_sb[0:1, :]
        idx_sb = load_const("idx_sb", idx_d[:], [128, total_cap // 16], I16)
        rel_sb = load_const("rel_sb", rel_d[:], [128, n_chunks], BF16)

        iotah_sb = const.tile([128, 128], BF16)
        nc.gpsimd.iota(iotah_sb[:], [[1, 128]], channel_multiplier=0,
                       allow_small_or_imprecise_dtypes=True)
        iota_sb = const.tile([128, 128], F32)
        nc.gpsimd.iota(iota_sb[:], [[1, 128]], channel_multiplier=0,
                       allow_small_or_imprecise_dtypes=True)
        iota_col = const.tile([128, 1], F32)
        nc.gpsimd.iota(iota_col[:], [[1, 1]], channel_multiplier=1,
                       allow_small_or_imprecise_dtypes=True)
        ident_sb = const.tile([128, 128], F32)
        nc.vector.tensor_scalar(ident_sb[:], iota_sb[:], iota_col[:], None,
                                op0=mybir.AluOpType.is_equal)

        hT = const.tile([128, npc], F32)  # h transposed, own nodes

        def alloc_rows():
            return dram.tile([npc, HID], BF16, tag="rows_d", name="rows_dram")

        def emit_block(rows_dram, b):
            """Transpose one updated hT block to row-major and DMA to DRAM."""
            blk = slice(b * 128, (b + 1) * 128)
            pt = ps_d.tile([128, 512], F32, tag="d")
            nc.tensor.transpose(pt[:, :128], hT[:, blk], ident_sb[:])
            r_sb = rpool.tile([128, 128], BF16, tag="r")
            nc.scalar.copy(r_sb[:], pt[:, :128])
            nc.sync.dma_start(out=rows_dram[blk, :], in_=r_sb[:])

        def allgather(rows_dram):
            hfull = dram.tile([cfg.n_pad, HID], BF16, tag="hfull_d", addr_space="Shared")
            nc.gpsimd.collective_compute(
                "AllGather", mybir.AluOpType.bypass,
                replica_groups=[list(range(C))],
                ins=[rows_dram.opt()], outs=[hfull.opt()],
            )
            return hfull

        # ---- embedding: hT = emb_W^T @ at_own + emb_b (mask-broadcast) ----
        with tc.tile_pool(name="atp", bufs=1) as atp:
            at_sb = atp.tile([FEAT, npc], F32, name="at_sb")
            nc.sync.dma_start(out=at_sb[:], in_=at_d[:])
            maskf_sb = atp.tile([1, npc], F32, name="maskf_sb")
            nc.vector.tensor_copy(maskf_sb[:], mask_sb[:])
            for j0 in range(0, npc, 512):
                jn = min(512, npc - j0)
                pe = ps_d.tile([128, 512], F32, tag="d", name="pe")
                nc.tensor.matmul(pe[:, :jn], embW_sb[:, :], at_sb[:, j0:j0 + jn],
                                 start=True, stop=False)
                nc.tensor.matmul(pe[:, :jn], embb_sb[:, :], maskf_sb[:, j0:j0 + jn],
                                 start=False, stop=True)
                nc.scalar.copy(hT[:, j0:j0 + jn], pe[:, :jn])
        rows0 = alloc_rows()
        for b in range(nb):
            emit_block(rows0, b)
        hfull = allgather(rows0)

        # ---- message passing layers ----
        for l in range(L):
            psums = {}
            rows_dram = alloc_rows() if l < L - 1 else None
            for ci, (nch, off16, chunk0, k) in enumerate(calls):
                cc = nch * 128
                g = gpool.tile([128, cfg.gcap // 128, 128], BF16, tag="g")
                nc.gpsimd.dma_gather(
                    g[:, :nch, :],
                    hfull[k * cfg.bank_rows:(k + 1) * cfg.bank_rows, :],
                    idx_sb[:, off16: off16 + cc // 16],
                    num_idxs=cc, num_idxs_reg=cc, elem_size=HID,
                    queue_num=ci % cfg.n_queues,
                )
                oh = ohpool.tile([128, cfg.gcap // 128, 128], BF16, tag="oh")
                nc.vector.tensor_tensor(
                    oh[:, :nch, :],
                    iotah_sb[:, :].unsqueeze(1).broadcast_to([128, nch, 128]),
                    rel_sb[:, chunk0:chunk0 + nch].unsqueeze(2).broadcast_to([128, nch, 128]),
                    op=mybir.AluOpType.is_equal)
                for t in range(nch):
                    b = chunk_block[chunk0 + t]
                    if first_chunk[b] == chunk0 + t:
                        psums[b] = ps_s.tile([128, 128], F32, tag="ps_s",
                                             name=f"psum_{b % 16}")
                    nc.tensor.matmul(psums[b][:], g[:, t, :], oh[:, t, :],
                                     start=(first_chunk[b] == chunk0 + t),
                                     stop=(last_chunk[b] == chunk0 + t))
                # when this call closes a super-block's bank-1 run, flush its
                # blocks' dense ops
                next_chunk0 = (calls[ci + 1][2] if ci + 1 < len(calls)
                               else n_chunks)
                done_blocks = [b for b in list(psums)
                               if last_chunk[b] < next_chunk0]
                for b in sorted(done_blocks):
                    blk = slice(b * 128, (b + 1) * 128)
                    s_sb = spool.tile([128, 128], BF16, tag="s")
                    nc.scalar.copy(s_sb[:], psums.pop(b)[:])
                    hTb_sb = spool.tile([128, 128], BF16, tag="s", name="hTb_sb")
                    nc.scalar.copy(hTb_sb[:], hT[:, blk])
                    hdeg_sb = spool.tile([128, 128], BF16, tag="s", name="hdeg_sb")
                    nc.vector.tensor_mul(hdeg_sb[:], hTb_sb[:], degb_sb[:, blk])
                    pa = ps_d.tile([128, 512], F32, tag="d")
                    nc.tensor.matmul(pa[:, :128], mtop_sb[l][:], s_sb[:], start=True, stop=False)
                    nc.tensor.matmul(pa[:, :128], mbot_sb[l][:], hdeg_sb[:], start=False, stop=False)
                    nc.tensor.matmul(pa[:, :128], mb_sb[l][:], deg_sb[:, blk], start=False, stop=True)
                    agg_sb = spool.tile([128, 128], BF16, tag="s")
                    nc.scalar.copy(agg_sb[:], pa[:, :128])

                    pu = ps_d.tile([128, 512], F32, tag="d")
                    nc.tensor.matmul(pu[:, :128], utop_sb[l][:], hTb_sb[:], start=True, stop=False)
                    nc.tensor.matmul(pu[:, :128], ubot_sb[l][:], agg_sb[:], start=False, stop=False)
                    nc.tensor.matmul(pu[:, :128], ub_sb[l][:], mask_sb[:, blk], start=False, stop=True)
                    relu_sb = spool.tile([128, 128], F32, tag="s")
                    nc.scalar.activation(relu_sb[:], pu[:, :128],
                                         mybir.ActivationFunctionType.Relu)
                    nc.vector.tensor_add(hT[:, blk], relu_sb[:], hT[:, blk])
                    if rows_dram is not None:
                        emit_block(rows_dram, b)
            assert not psums
            # blocks with zero chunks (no in-edges anywhere): still need dense
            for b in range(nb):
                if b in first_chunk:
                    continue
                blk = slice(b * 128, (b + 1) * 128)
                hTb_sb = spool.tile([128, 128], BF16, tag="s", name="hTb_sb")
                nc.scalar.copy(hTb_sb[:], hT[:, blk])
                pu = ps_d.tile([128, 512], F32, tag="d")
                nc.tensor.matmul(pu[:, :128], utop_sb[l][:], hTb_sb[:], start=True, stop=False)
                nc.tensor.matmul(pu[:, :128], ub_sb[l][:], mask_sb[:, blk], start=False, stop=True)
                relu_sb = spool.tile([128, 128], F32, tag="s")
                nc.scalar.activation(relu_sb[:], pu[:, :128],
                                     mybir.ActivationFunctionType.Relu)
                nc.vector.tensor_add(hT[:, blk], relu_sb[:], hT[:, blk])
                if rows_dram is not None:
                    emit_block(rows_dram, b)
            if rows_dram is not None:
                hfull = allgather(rows_dram)

        # ---- readout: g = mean(h) ; out = MLP(g) ----
        part_sb = spool.tile([128, 1], F32, tag="s")
        nc.vector.tensor_reduce(part_sb[:], hT[:, :], axis=mybir.AxisListType.X,
                                op=mybir.AluOpType.add)
        part_dram = dram.tile([128, 1], F32, tag="pt_d")
        gsum_dram = dram.tile([128, 1], F32, tag="gs_d", addr_space="Shared")
        nc.sync.dma_start(out=part_dram[:], in_=part_sb[:])
        nc.gpsimd.collective_compute(
            "AllReduce", mybir.AluOpType.add,
            replica_groups=[list(range(C))],
            ins=[part_dram.opt()], outs=[gsum_dram.opt()],
        )
        gsum_sb = spool.tile([128, 1], F32, tag="s")
        nc.sync.dma_start(out=gsum_sb[:], in_=gsum_dram[:])

        p1 = ps_d.tile([128, 512], F32, tag="d")
        nc.tensor.matmul(p1[:HID // 2, :1], ow1_sb[:], gsum_sb[:], start=True, stop=True)
        o1_sb = spool.tile([HID // 2, 1], F32, tag="o1")
        nc.scalar.activation(o1_sb[:], p1[:HID // 2, :1],
                             mybir.ActivationFunctionType.Relu,
                             bias=ob1_sb[:], scale=1.0 / cfg.n_nodes)
        p2 = ps_d.tile([128, 512], F32, tag="d")
        nc.tensor.matmul(p2[:HID // 4, :1], ow2_sb[:], o1_sb[:], start=True, stop=True)
        o2_sb = spool.tile([HID // 4, 1], F32, tag="o2")
        nc.scalar.activation(o2_sb[:], p2[:HID // 4, :1],
                             mybir.ActivationFunctionType.Relu, bias=ob2_sb[:])
        p3 = ps_d.tile([128, 512], F32, tag="d")
        nc.tensor.matmul(p3[:1, :1], ow3_sb[:], o2_sb[:], start=True, stop=True)
        o3_sb = spool.tile([1, 1], F32, tag="o3")
        nc.scalar.activation(o3_sb[:], p3[:1, :1],
                             mybir.ActivationFunctionType.Identity, bias=ob3_sb[:])
        nc.sync.dma_start(out=out_d[:], in_=o3_sb[:])

    nc.compile()
    return nc


def make_in_maps(cfg: Cfg, inputs, per_core_arr):
    C, npc, FEAT, HID, L = cfg.n_cores, cfg.npc, cfg.feat, cfg.hid, cfg.layers
    af = np.asarray(inputs["atom_features"], np.float32)
    at_pad = np.zeros((cfg.n_pad, FEAT), np.float32)
    at_pad[:cfg.n_nodes] = af
    msg_W = np.asarray(inputs["msg_W"], np.float32)
    upd_W = np.asarray(inputs["upd_W"], np.float32)
    shared = dict(
        emb_W=np.asarray(inputs["emb_W"], np.float32),
        emb_b=np.asarray(inputs["emb_b"], np.float32)[None, :],
        msg_top=msg_W[:, :HID, :].reshape(L * HID, HID).astype(BF),
        msg_bot=msg_W[:, HID:, :].reshape(L * HID, HID).astype(BF),
        msg_bias=np.asarray(inputs["msg_b"], np.float32).astype(BF),
        upd_top=upd_W[:, :HID, :].reshape(L * HID, HID).astype(BF),
        upd_bot=upd_W[:, HID:, :].reshape(L * HID, HID).astype(BF),
        upd_bias=np.asarray(inputs["upd_b"], np.float32).astype(BF),
        out_W1=np.asarray(inputs["out_W1"], np.float32),
        out_b1=np.asarray(inputs["out_b1"], np.float32)[:, None],
        out_W2=np.asarray(inputs["out_W2"], np.float32),
        out_b2=np.asarray(inputs["out_b2"], np.float32)[:, None],
        out_W3=np.asarray(inputs["out_W3"], np.float32),
        out_b3=np.asarray(inputs["out_b3"], np.float32)[:, None],
    )
    in_maps = []
    for c in range(C):
        d = dict(shared)
        d["at_own"] = at_pad[c * npc:(c + 1) * npc].T.copy()
        d.update(per_core_arr[c])
        in_maps.append(d)
    return in_maps


_prog_cache = {}


def run(cfg: Cfg, inputs, trace=False):
    meta, per_core_arr = plan_edges(cfg, np.asarray(inputs["edge_indices"]))
    key = (cfg, hashlib.sha1(meta["caps"].tobytes()).hexdigest())
    if key not in _prog_cache:
        _prog_cache[key] = build_program(cfg, meta)
    nc = _prog_cache[key]
    in_maps = make_in_maps(cfg, inputs, per_core_arr)
    res = run_bass_kernel_spmd(nc, in_maps, core_ids=list(range(cfg.n_cores)),
                               trace=trace)
    out = res.results[0]["result"].astype(np.float32)
    return out, res


def kernel(**inputs) -> np.ndarray:
    out, _ = run(Cfg(), inputs)
    return out


# revision 36
# speedup vs baseline: 1.3751x; 1.0453x over previous
"""Trainium2 Bass kernel for EnhancedGNNModel (3-layer GNN message passing).

Strategy (8 NeuronCores, SPMD):
  - Nodes are sharded by dst range: core c owns rows [c*NPC, (c+1)*NPC).
  - Edges are sharded by dst: each core aggregates messages for its own nodes.
  - Per-edge message Linear(concat(h_src, h_dst)) @ W is split algebraically:
        msgs = h[src] @ W_top + h[dst] @ W_bot + b
    so segment_sum(msgs, dst) = (sum_{e->d} h[src]) @ W_top
                                + deg[d] * (h[d] @ W_bot) + deg[d] * b.
    Only S[d] = sum of gathered h[src] rows needs edge-granularity work.
  - Edges are grouped per (super-block of SB dst blocks, src bank); one
    dma_gather call per group pulls h[src] rows into a [128, nch, 128] SBUF
    tile; one bulk is_equal builds all nch one-hot matrices at once; per-chunk
    PE matmuls accumulate S^T per dst block in PSUM.
  - h lives transposed ([HID, node]) in SBUF for all dense matmuls; dense
    weights run in bf16. Updated rows are transposed back via the PE, written
    to DRAM, and AllGathered so every core has the full h for the next
    layer's gathers.
"""
import hashlib
import ml_dtypes
import numpy as np
from contextlib import ExitStack
from dataclasses import dataclass

import concourse.bass as bass
import concourse.tile as tile
from concourse import bacc, mybir
from concourse.bass_utils import run_bass_kernel_spmd

F32 = mybir.dt.float32
BF16 = mybir.dt.bfloat16
I16 = mybir.dt.int16
BF = ml_dtypes.bfloat16


@dataclass(frozen=True)
class Cfg:
    n_nodes: int = 50000
    n_edges: int = 800000
    feat: int = 64
    hid: int = 128
    layers: int = 3
    n_cores: int = 8
    sb: int = 5  # dst blocks per super-block (one gather group per bank)
    gcap: int = 1024  # max indices per dma_gather call (ring-capacity limit)
    dma_scratch: int = 16384
    n_queues: int = 1

    @property
    def npc(self):  # nodes per core (multiple of 128)
        per = -(-self.n_nodes // self.n_cores)
        return -(-per // 128) * 128

    @property
    def n_pad(self):
        return self.npc * self.n_cores

    @property
    def rows_a(self):  # per-core rows in bank A (blocks 0..rows_a/128-1)
        r = (self.n_blocks // 2) * 128
        assert r * self.n_cores <= 32768, "dma_gather int16 index limit"
        assert (self.npc - r) * self.n_cores <= 32768
        return r

    @property
    def n_blocks(self):
        return self.npc // 128


def plan_edges(cfg: Cfg, edge_indices: np.ndarray):
    """Host-side sharding plan. Edge order per core:
    super-block s -> bank k -> block b -> edges (padded to 128 per (b,k)).
    Returns (meta, per_core). meta['calls'] is a list of
    (nch, off16, chunk0, bank) and meta['chunk_block'] maps chunk -> block.
    """
    src = np.asarray(edge_indices[0], dtype=np.int64)
    dst = np.asarray(edge_indices[1], dtype=np.int64)
    C, npc, nb = cfg.n_cores, cfg.npc, cfg.n_blocks
    ra, rb = cfg.rows_a, cfg.npc - cfg.rows_a

    core = dst // npc
    ldst = dst - core * npc
    block = ldst >> 7
    dst_rel = ldst & 127
    core_s = src // npc
    r_s = src - core_s * npc
    bank = (r_s >= ra).astype(np.int64)
    bidx = np.where(bank == 0, core_s * ra + r_s, core_s * rb + (r_s - ra))

    counts = np.zeros((C, nb, 2), dtype=np.int64)
    np.add.at(counts, (core, block, bank), 1)
    caps = counts.max(axis=0)  # [nb, 2]
    caps = np.where(caps > 0, ((caps + 127) // 128) * 128, 0).astype(np.int64)

    key = (core * nb + block) * 2 + bank
    order = np.argsort(key, kind="stable")
    bidx_s, rel_s = bidx[order], dst_rel[order]
    group_sizes = np.bincount(key[order], minlength=C * nb * 2)
    group_starts = np.concatenate([[0], np.cumsum(group_sizes)])

    # super-block partition of blocks
    sblocks = [list(range(s, min(s + cfg.sb, nb)))
               for s in range(0, nb, cfg.sb)]

    # static layout: order of (s, k, b) groups; each group caps[b,k] slots
    layout = []  # (b, k, cap)
    for bs in sblocks:
        for k in range(2):
            for b in bs:
                if caps[b, k]:
                    layout.append((b, k, int(caps[b, k])))
    total_cap = sum(g[2] for g in layout)
    n_chunks = total_cap // 128

    # chunk -> block map and call list (calls split at gcap, never across
    # (s, k) boundaries; all chunks of one call share the bank)
    chunk_block = []
    calls = []  # (nch, off16, chunk0, bank)
    off = 0
    for bs in sblocks:
        for k in range(2):
            groups = [g for g in layout if g[0] in bs and g[1] == k]
            run = sum(g[2] for g in groups)
            if run == 0:
                continue
            for g in groups:
                chunk_block += [g[0]] * (g[2] // 128)
            q = 0
            while q < run:
                cc = min(cfg.gcap, run - q)
                calls.append((cc // 128, (off + q) // 16, (off + q) // 128, k))
                q += cc
            off += run
    assert off == total_cap and len(chunk_block) == n_chunks

    per_core_arr = []
    for c in range(C):
        idx_flat = np.zeros(total_cap, dtype=np.int16)
        rel_flat = np.full(total_cap, -1.0, dtype=np.float32)
        pos = 0
        for (b, k, cap) in layout:
            g = (c * nb + b) * 2 + k
            s0, n = group_starts[g], group_sizes[g]
            idx_flat[pos:pos + n] = bidx_s[s0:s0 + n].astype(np.int16)
            rel_flat[pos:pos + n] = rel_s[s0:s0 + n].astype(np.float32)
            pos += cap
        assert pos == total_cap

        # wrap indices per call: idx j of a call sits at [j % 16, j // 16]
        idx_w = np.zeros((16, total_cap // 16), dtype=np.int16)
        for (nch, off16, _, _) in calls:
            cc = nch * 128
            seg = idx_flat[off16 * 16: off16 * 16 + cc].reshape(cc // 16, 16).T
            idx_w[:, off16: off16 + cc // 16] = seg
        idx_w = np.tile(idx_w, (8, 1))
        rel_t = rel_flat.reshape(n_chunks, 128).T.astype(BF)  # [128, n_chunks]

        deg = np.zeros(npc, dtype=np.float32)
        m = core == c
        np.add.at(deg, ldst[m], 1.0)
        mask = np.zeros(npc, dtype=np.float32)
        lo = c * npc
        mask[: max(0, min(npc, cfg.n_nodes - lo))] = 1.0
        per_core_arr.append(dict(idxs=idx_w, dstrel=rel_t,
                                 mask=mask[None, :].astype(BF),
                                 deg_bcast=np.tile(deg[None, :], (128, 1)).astype(BF)))

    meta = dict(caps=caps, calls=calls, chunk_block=chunk_block,
                sblocks=sblocks, n_chunks=n_chunks, total_cap=total_cap)
    return meta, per_core_arr


def build_program(cfg: Cfg, meta):
    C, npc, nb = cfg.n_cores, cfg.npc, cfg.n_blocks
    FEAT, HID, L = cfg.feat, cfg.hid, cfg.layers
    n_chunks, total_cap = meta["n_chunks"], meta["total_cap"]
    calls, chunk_block, sblocks = meta["calls"], meta["chunk_block"], meta["sblocks"]
    max_nch = max(c[0] for c in calls)

    # first/last chunk index per block (chunks of one block are contiguous
    # except for the bank split; find global first/last over all its chunks)
    first_chunk = {}
    last_chunk = {}
    for t, b in enumerate(chunk_block):
        first_chunk.setdefault(b, t)
        last_chunk[b] = t

    nc = bacc.Bacc("TRN2", target_bir_lowering=False, debug=False, num_devices=C,
                   dynamic_dma_scratch_size=cfg.dma_scratch,
                   num_swdge_queues=cfg.n_queues)

    def inp(name, shape, dt=F32):
        return nc.dram_tensor(name, shape, dt, kind="ExternalInput").ap()

    at_d = inp("at_own", [FEAT, npc])
    embW_d = inp("emb_W", [FEAT, HID])
    embb_d = inp("emb_b", [1, HID])
    mtop_d = inp("msg_top", [L * HID, HID], BF16)
    mbot_d = inp("msg_bot", [L * HID, HID], BF16)
    mb_d = inp("msg_bias", [L, HID], BF16)
    utop_d = inp("upd_top", [L * HID, HID], BF16)
    ubot_d = inp("upd_bot", [L * HID, HID], BF16)
    ub_d = inp("upd_bias", [L, HID], BF16)
    ow1_d = inp("out_W1", [HID, HID // 2])
    ob1_d = inp("out_b1", [HID // 2, 1])
    ow2_d = inp("out_W2", [HID // 2, HID // 4])
    ob2_d = inp("out_b2", [HID // 4, 1])
    ow3_d = inp("out_W3", [HID // 4, 1])
    ob3_d = inp("out_b3", [1, 1])
    mask_d = inp("mask", [1, npc], BF16)
    degb_d = inp("deg_bcast", [128, npc], BF16)
    idx_d = inp("idxs", [128, total_cap // 16], I16)
    rel_d = inp("dstrel", [128, n_chunks], BF16)
    out_d = nc.dram_tensor("result", [1, 1], F32, kind="ExternalOutput").ap()

    with tile.TileContext(nc) as tc, ExitStack() as ctx:
        const = ctx.enter_context(tc.tile_pool(name="const", bufs=1))
        gpool = ctx.enter_context(tc.tile_pool(name="g", bufs=2))
        ohpool = ctx.enter_context(tc.tile_pool(name="oh", bufs=2))
        spool = ctx.enter_context(tc.tile_pool(name="s", bufs=8))
        rpool = ctx.enter_context(tc.tile_pool(name="rows", bufs=4))
        ps_s = ctx.enter_context(tc.tile_pool(name="ps_s", bufs=6, space="PSUM"))
        ps_d = ctx.enter_context(tc.tile_pool(name="ps_d", bufs=2, space="PSUM"))
        dram = ctx.enter_context(tc.tile_pool(name="dram", bufs=2, space="DRAM"))

        def load_const(name, ap_dram, shape, dt=F32):
            t = const.tile(shape, dt, name=name, tag=name)
            nc.sync.dma_start(out=t[:], in_=ap_dram)
            return t

        embW_sb = load_const("embW_sb", embW_d[:], [FEAT, HID])
        embb_sb = load_const("embb_sb", embb_d[:], [1, HID])
        mtop_sb = [load_const(f"mtop{l}", mtop_d[l * HID:(l + 1) * HID, :], [HID, HID], BF16) for l in range(L)]
        mbot_sb = [load_const(f"mbot{l}", mbot_d[l * HID:(l + 1) * HID, :], [HID, HID], BF16) for l in range(L)]
        mb_sb = [load_const(f"mb{l}", mb_d[l:l + 1, :], [1, HID], BF16) for l in range(L)]
        utop_sb = [load_const(f"utop{l}", utop_d[l * HID:(l + 1) * HID, :], [HID, HID], BF16) for l in range(L)]
        ubot_sb = [load_const(f"ubot{l}", ubot_d[l * HID:(l + 1) * HID, :], [HID, HID], BF16) for l in range(L)]
        ub_sb = [load_const(f"ub{l}", ub_d[l:l + 1, :], [1, HID], BF16) for l in range(L)]
        ow1_sb = load_const("ow1_sb", ow1_d[:], [HID, HID // 2])
        ob1_sb = load_const("ob1_sb", ob1_d[:], [HID // 2, 1])
        ow2_sb = load_const("ow2_sb", ow2_d[:], [HID // 2, HID // 4])
        ob2_sb = load_const("ob2_sb", ob2_d[:], [HID // 4, 1])
        ow3_sb = load_const("ow3_sb", ow3_d[:], [HID // 4, 1])
        ob3_sb = load_const("ob3_sb", ob3_d[:], [1, 1])
        mask_sb = load_const("mask_sb", mask_d[:], [1, npc], BF16)
        degb_sb = load_const("degb_sb", degb_d[:], [128, npc], BF16)
        deg_sb = degb_sb[0:1, :]
        idx_sb = load_const("idx_sb", idx_d[:], [128, total_cap // 16], I16)
        rel_sb = load_const("rel_sb", rel_d[:], [128, n_chunks], BF16)

        iotah_sb = const.tile([128, 128], BF16)
        nc.gpsimd.iota(iotah_sb[:], [[1, 128]], channel_multiplier=0,
                       allow_small_or_imprecise_dtypes=True)
        iota_sb = const.tile([128, 128], F32)
        nc.gpsimd.iota(iota_sb[:], [[1, 128]], channel_multiplier=0,
                       allow_small_or_imprecise_dtypes=True)
        iota_col = const.tile([128, 1], F32)
        nc.gpsimd.iota(iota_col[:], [[1, 1]], channel_multiplier=1,
                       allow_small_or_imprecise_dtypes=True)
        ident_sb = const.tile([128, 128], F32)
        nc.vector.tensor_scalar(ident_sb[:], iota_sb[:], iota_col[:], None,
                                op0=mybir.AluOpType.is_equal)

        hT = const.tile([128, npc], F32)  # h transposed, own nodes

        def alloc_rows():
            return dram.tile([npc, HID], BF16, tag="rows_d", name="rows_dram")

        def emit_block(rows_dram, b):
            """Transpose one updated hT block to row-major and DMA to DRAM."""
            blk = slice(b * 128, (b + 1) * 128)
            pt = ps_d.tile([128, 512], F32, tag="d")
            nc.tensor.transpose(pt[:, :128], hT[:, blk], ident_sb[:])
            r_sb = rpool.tile([128, 128], BF16, tag="r")
            nc.scalar.copy(r_sb[:], pt[:, :128])
            nc.sync.dma_start(out=rows_dram[blk, :], in_=r_sb[:])

        ra, rb = cfg.rows_a, npc - cfg.rows_a

        def allgather_half(rows_dram, k):
            rows = rows_dram[0:ra, :] if k == 0 else rows_dram[ra:npc, :]
            n = ra if k == 0 else rb
            hf = dram.tile([C * n, HID], BF16, tag=f"hfull{k}_d",
                           addr_space="Shared", name="hf")
            nc.gpsimd.collective_compute(
                "AllGather", mybir.AluOpType.bypass,
                replica_groups=[list(range(C))],
                ins=[rows.opt()], outs=[hf.opt()],
            )
            return hf

        # ---- embedding: hT = emb_W^T @ at_own + emb_b (mask-broadcast) ----
        with tc.tile_pool(name="atp", bufs=1) as atp:
            at_sb = atp.tile([FEAT, npc], F32, name="at_sb")
            nc.sync.dma_start(out=at_sb[:], in_=at_d[:])
            maskf_sb = atp.tile([1, npc], F32, name="maskf_sb")
            nc.vector.tensor_copy(maskf_sb[:], mask_sb[:])
            for j0 in range(0, npc, 512):
                jn = min(512, npc - j0)
                pe = ps_d.tile([128, 512], F32, tag="d", name="pe")
                nc.tensor.matmul(pe[:, :jn], embW_sb[:, :], at_sb[:, j0:j0 + jn],
                                 start=True, stop=False)
                nc.tensor.matmul(pe[:, :jn], embb_sb[:, :], maskf_sb[:, j0:j0 + jn],
                                 start=False, stop=True)
                nc.scalar.copy(hT[:, j0:j0 + jn], pe[:, :jn])
        rows0 = alloc_rows()
        hfull = [None, None]
        for b in range(nb):
            emit_block(rows0, b)
            if b == ra // 128 - 1:
                hfull[0] = allgather_half(rows0, 0)
        hfull[1] = allgather_half(rows0, 1)

        # ---- message passing layers ----
        for l in range(L):
            psums = {}
            rows_dram = alloc_rows() if l < L - 1 else None
            next_hfull = [None, None]
            flushed = 0
            for ci, (nch, off16, chunk0, k) in enumerate(calls):
                cc = nch * 128
                g = gpool.tile([128, cfg.gcap // 128, 128], BF16, tag="g")
                nc.gpsimd.dma_gather(
                    g[:, :nch, :],
                    hfull[k][:, :],
                    idx_sb[:, off16: off16 + cc // 16],
                    num_idxs=cc, num_idxs_reg=cc, elem_size=HID,
                    queue_num=ci % cfg.n_queues,
                )
                oh = ohpool.tile([128, cfg.gcap // 128, 128], BF16, tag="oh")
                nc.vector.tensor_tensor(
                    oh[:, :nch, :],
                    iotah_sb[:, :].unsqueeze(1).broadcast_to([128, nch, 128]),
                    rel_sb[:, chunk0:chunk0 + nch].unsqueeze(2).broadcast_to([128, nch, 128]),
                    op=mybir.AluOpType.is_equal)
                for t in range(nch):
                    b = chunk_block[chunk0 + t]
                    if first_chunk[b] == chunk0 + t:
                        psums[b] = ps_s.tile([128, 128], F32, tag="ps_s",
                                             name=f"psum_{b % 16}")
                    nc.tensor.matmul(psums[b][:], g[:, t, :], oh[:, t, :],
                                     start=(first_chunk[b] == chunk0 + t),
                                     stop=(last_chunk[b] == chunk0 + t))
                # when this call closes a super-block's bank-1 run, flush its
                # blocks' dense ops
                next_chunk0 = (calls[ci + 1][2] if ci + 1 < len(calls)
                               else n_chunks)
                done_blocks = [b for b in list(psums)
                               if last_chunk[b] < next_chunk0]
                for b in sorted(done_blocks):
                    blk = slice(b * 128, (b + 1) * 128)
                    s_sb = spool.tile([128, 128], BF16, tag="s")
                    nc.scalar.copy(s_sb[:], psums.pop(b)[:])
                    hTb_sb = spool.tile([128, 128], BF16, tag="s", name="hTb_sb")
                    nc.scalar.copy(hTb_sb[:], hT[:, blk])
                    hdeg_sb = spool.tile([128, 128], BF16, tag="s", name="hdeg_sb")
                    nc.vector.tensor_mul(hdeg_sb[:], hTb_sb[:], degb_sb[:, blk])
                    pa = ps_d.tile([128, 512], F32, tag="d")
                    nc.tensor.matmul(pa[:, :128], mtop_sb[l][:], s_sb[:], start=True, stop=False)
                    nc.tensor.matmul(pa[:, :128], mbot_sb[l][:], hdeg_sb[:], start=False, stop=False)
                    nc.tensor.matmul(pa[:, :128], mb_sb[l][:], deg_sb[:, blk], start=False, stop=True)
                    agg_sb = spool.tile([128, 128], BF16, tag="s")
                    nc.scalar.copy(agg_sb[:], pa[:, :128])

                    pu = ps_d.tile([128, 512], F32, tag="d")
                    nc.tensor.matmul(pu[:, :128], utop_sb[l][:], hTb_sb[:], start=True, stop=False)
                    nc.tensor.matmul(pu[:, :128], ubot_sb[l][:], agg_sb[:], start=False, stop=False)
                    nc.tensor.matmul(pu[:, :128], ub_sb[l][:], mask_sb[:, blk], start=False, stop=True)
                    relu_sb = spool.tile([128, 128], F32, tag="s")
                    nc.scalar.activation(relu_sb[:], pu[:, :128],
                                         mybir.ActivationFunctionType.Relu)
                    nc.vector.tensor_add(hT[:, blk], relu_sb[:], hT[:, blk])
                    if rows_dram is not None:
                        emit_block(rows_dram, b)
                        flushed += 1
                        if flushed == ra // 128:
                            next_hfull[0] = allgather_half(rows_dram, 0)
            assert not psums
            # blocks with zero chunks (no in-edges anywhere): still need dense
            for b in range(nb):
                if b in first_chunk:
                    continue
                blk = slice(b * 128, (b + 1) * 128)
                hTb_sb = spool.tile([128, 128], BF16, tag="s", name="hTb_sb")
                nc.scalar.copy(hTb_sb[:], hT[:, blk])
                pu = ps_d.tile([128, 512], F32, tag="d")
                nc.tensor.matmul(pu[:, :128], utop_sb[l][:], hTb_sb[:], start=True, stop=False)
                nc.tensor.matmul(pu[:, :128], ub_sb[l][:], mask_sb[:, blk], start=False, stop=True)
                relu_sb = spool.tile([128, 128], F32, tag="s")
                nc.scalar.activation(relu_sb[:], pu[:, :128],
                                     mybir.ActivationFunctionType.Relu)
                nc.vector.tensor_add(hT[:, blk], relu_sb[:], hT[:, blk])
                if rows_dram is not None:
                    emit_block(rows_dram, b)
                    flushed += 1
                    if flushed == ra // 128:
                        next_hfull[0] = allgather_half(rows_dram, 0)
            if rows_dram is not None:
                next_hfull[1] = allgather_half(rows_dram, 1)
                assert next_hfull[0] is not None
                hfull = next_hfull

        # ---- readout: g = mean(h) ; out = MLP(g) ----
        part_sb = spool.tile([128, 1], F32, tag="s")
        nc.vector.tensor_reduce(part_sb[:], hT[:, :], axis=mybir.AxisListType.X,
                                op=mybir.AluOpType.add)
        part_dram = dram.tile([128, 1], F32, tag="pt_d")
        gsum_dram = dram.tile([128, 1], F32, tag="gs_d", addr_space="Shared")
        nc.sync.dma_start(out=part_dram[:], in_=part_sb[:])
        nc.gpsimd.collective_compute(
            "AllReduce", mybir.AluOpType.add,
            replica_groups=[list(range(C))],
            ins=[part_dram.opt()], outs=[gsum_dram.opt()],
        )
        gsum_sb = spool.tile([128, 1], F32, tag="s")
        nc.sync.dma_start(out=gsum_sb[:], in_=gsum_dram[:])

        p1 = ps_d.tile([128, 512], F32, tag="d")
        nc.tensor.matmul(p1[:HID // 2, :1], ow1_sb[:], gsum_sb[:], start=True, stop=True)
        o1_sb = spool.tile([HID // 2, 1], F32, tag="o1")
        nc.scalar.activation(o1_sb[:], p1[:HID // 2, :1],
                             mybir.ActivationFunctionType.Relu,
                             bias=ob1_sb[:], scale=1.0 / cfg.n_nodes)
        p2 = ps_d.tile([128, 512], F32, tag="d")
        nc.tensor.matmul(p2[:HID // 4, :1], ow2_sb[:], o1_sb[:], start=True, stop=True)
        o2_sb = spool.tile([HID // 4, 1], F32, tag="o2")
        nc.scalar.activation(o2_sb[:], p2[:HID // 4, :1],
                             mybir.ActivationFunctionType.Relu, bias=ob2_sb[:])
        p3 = ps_d.tile([128, 512], F32, tag="d")
        nc.tensor.matmul(p3[:1, :1], ow3_sb[:], o2_sb[:], start=True, stop=True)
        o3_sb = spool.tile([1, 1], F32, tag="o3")
        nc.scalar.activation(o3_sb[:], p3[:1, :1],
                             mybir.ActivationFunctionType.Identity, bias=ob3_sb[:])
        nc.sync.dma_start(out=out_d[:], in_=o3_sb[:])

    nc.compile()
    return nc


def make_in_maps(cfg: Cfg, inputs, per_core_arr):
    C, npc, FEAT, HID, L = cfg.n_cores, cfg.npc, cfg.feat, cfg.hid, cfg.layers
    af = np.asarray(inputs["atom_features"], np.float32)
    at_pad = np.zeros((cfg.n_pad, FEAT), np.float32)
    at_pad[:cfg.n_nodes] = af
    msg_W = np.asarray(inputs["msg_W"], np.float32)
    upd_W = np.asarray(inputs["upd_W"], np.float32)
    shared = dict(
        emb_W=np.asarray(inputs["emb_W"], np.float32),
        emb_b=np.asarray(inputs["emb_b"], np.float32)[None, :],
        msg_top=msg_W[:, :HID, :].reshape(L * HID, HID).astype(BF),
        msg_bot=msg_W[:, HID:, :].reshape(L * HID, HID).astype(BF),
        msg_bias=np.asarray(inputs["msg_b"], np.float32).astype(BF),
        upd_top=upd_W[:, :HID, :].reshape(L * HID, HID).astype(BF),
        upd_bot=upd_W[:, HID:, :].reshape(L * HID, HID).astype(BF),
        upd_bias=np.asarray(inputs["upd_b"], np.float32).astype(BF),
        out_W1=np.asarray(inputs["out_W1"], np.float32),
        out_b1=np.asarray(inputs["out_b1"], np.float32)[:, None],
        out_W2=np.asarray(inputs["out_W2"], np.float32),
        out_b2=np.asarray(inputs["out_b2"], np.float32)[:, None],
        out_W3=np.asarray(inputs["out_W3"], np.float32),
        out_b3=np.asarray(inputs["out_b3"], np.float32)[:, None],
    )
    in_maps = []
    for c in range(C):
        d = dict(shared)
        d["at_own"] = at_pad[c * npc:(c + 1) * npc].T.copy()
        d.update(per_core_arr[c])
        in_maps.append(d)
    return in_maps


_prog_cache = {}


def run(cfg: Cfg, inputs, trace=False):
    meta, per_core_arr = plan_edges(cfg, np.asarray(inputs["edge_indices"]))
    key = (cfg, hashlib.sha1(meta["caps"].tobytes()).hexdigest())
    if key not in _prog_cache:
        _prog_cache[key] = build_program(cfg, meta)
    nc = _prog_cache[key]
    in_maps = make_in_maps(cfg, inputs, per_core_arr)
    res = run_bass_kernel_spmd(nc, in_maps, core_ids=list(range(cfg.n_cores)),
                               trace=trace)
    out = res.results[0]["result"].astype(np.float32)
    return out, res


def kernel(**inputs) -> np.ndarray:
    out, _ = run(Cfg(), inputs)
    return out


# revision 38
# speedup vs baseline: 1.4284x; 1.0388x over previous
"""Trainium2 Bass kernel for EnhancedGNNModel (3-layer GNN message passing).

Strategy (8 NeuronCores, SPMD):
  - Nodes are sharded by dst range: core c owns rows [c*NPC, (c+1)*NPC).
  - Edges are sharded by dst: each core aggregates messages for its own nodes.
  - Per-edge message Linear(concat(h_src, h_dst)) @ W is split algebraically:
        msgs = h[src] @ W_top + h[dst] @ W_bot + b
    so segment_sum(msgs, dst) = (sum_{e->d} h[src]) @ W_top
                                + deg[d] * (h[d] @ W_bot) + deg[d] * b.
    Only S[d] = sum of gathered h[src] rows needs edge-granularity work.
  - Edges are grouped per (super-block of SB dst blocks, src bank); one
    dma_gather call per group pulls h[src] rows into a [128, nch, 128] SBUF
    tile; one bulk is_equal builds all nch one-hot matrices at once; per-chunk
    PE matmuls accumulate S^T per dst block in PSUM.
  - h lives transposed ([HID, node]) in SBUF for all dense matmuls; dense
    weights run in bf16. Updated rows are transposed back via the PE, written
    to DRAM, and AllGathered so every core has the full h for the next
    layer's gathers.
"""
import hashlib
import ml_dtypes
import numpy as np
from contextlib import ExitStack
from dataclasses import dataclass

import concourse.bass as bass
import concourse.tile as tile
from concourse import bacc, mybir
from concourse.bass_utils import run_bass_kernel_spmd

F32 = mybir.dt.float32
BF16 = mybir.dt.bfloat16
I16 = mybir.dt.int16
BF = ml_dtypes.bfloat16


@dataclass(frozen=True)
class Cfg:
    n_nodes: int = 50000
    n_edges: int = 800000
    feat: int = 64
    hid: int = 128
    layers: int = 3
    n_cores: int = 8
    sb: int = 5  # dst blocks per super-block (one gather group per bank)
    gcap: int = 1024  # max indices per dma_gather call (ring-capacity limit)
    dma_scratch: int = 16384
    n_queues: int = 1

    @property
    def npc(self):  # nodes per core (multiple of 128)
        per = -(-self.n_nodes // self.n_cores)
        return -(-per // 128) * 128

    @property
    def n_pad(self):
        return self.npc * self.n_cores

    @property
    def rows_a(self):  # per-core rows in bank A (blocks 0..rows_a/128-1)
        # as large as the int16 gather-index limit allows: AG-A is issued
        # mid-layer (hideable), so the layer-boundary wait is only AG-B
        r = min((32768 // self.n_cores) // 128 * 128, self.npc - 128)
        assert r * self.n_cores <= 32768, "dma_gather int16 index limit"
        assert (self.npc - r) * self.n_cores <= 32768
        return r

    @property
    def n_blocks(self):
        return self.npc // 128


def plan_edges(cfg: Cfg, edge_indices: np.ndarray):
    """Host-side sharding plan. Edge order per core:
    super-block s -> bank k -> block b -> edges (padded to 128 per (b,k)).
    Returns (meta, per_core). meta['calls'] is a list of
    (nch, off16, chunk0, bank) and meta['chunk_block'] maps chunk -> block.
    """
    src = np.asarray(edge_indices[0], dtype=np.int64)
    dst = np.asarray(edge_indices[1], dtype=np.int64)
    C, npc, nb = cfg.n_cores, cfg.npc, cfg.n_blocks
    ra, rb = cfg.rows_a, cfg.npc - cfg.rows_a

    core = dst // npc
    ldst = dst - core * npc
    block = ldst >> 7
    dst_rel = ldst & 127
    core_s = src // npc
    r_s = src - core_s * npc
    bank = (r_s >= ra).astype(np.int64)
    bidx = np.where(bank == 0, core_s * ra + r_s, core_s * rb + (r_s - ra))

    counts = np.zeros((C, nb, 2), dtype=np.int64)
    np.add.at(counts, (core, block, bank), 1)
    caps = counts.max(axis=0)  # [nb, 2]
    caps = np.where(caps > 0, ((caps + 127) // 128) * 128, 0).astype(np.int64)

    key = (core * nb + block) * 2 + bank
    order = np.argsort(key, kind="stable")
    bidx_s, rel_s = bidx[order], dst_rel[order]
    group_sizes = np.bincount(key[order], minlength=C * nb * 2)
    group_starts = np.concatenate([[0], np.cumsum(group_sizes)])

    # super-block partition of blocks
    sblocks = [list(range(s, min(s + cfg.sb, nb)))
               for s in range(0, nb, cfg.sb)]

    # static layout: order of (s, k, b) groups; each group caps[b,k] slots
    layout = []  # (b, k, cap)
    for bs in sblocks:
        for k in range(2):
            for b in bs:
                if caps[b, k]:
                    layout.append((b, k, int(caps[b, k])))
    total_cap = sum(g[2] for g in layout)
    n_chunks = total_cap // 128

    # chunk -> block map and call list (calls split at gcap, never across
    # (s, k) boundaries; all chunks of one call share the bank)
    chunk_block = []
    calls = []  # (nch, off16, chunk0, bank)
    off = 0
    for bs in sblocks:
        for k in range(2):
            groups = [g for g in layout if g[0] in bs and g[1] == k]
            run = sum(g[2] for g in groups)
            if run == 0:
                continue
            for g in groups:
                chunk_block += [g[0]] * (g[2] // 128)
            q = 0
            while q < run:
                cc = min(cfg.gcap, run - q)
                calls.append((cc // 128, (off + q) // 16, (off + q) // 128, k))
                q += cc
            off += run
    assert off == total_cap and len(chunk_block) == n_chunks

    per_core_arr = []
    for c in range(C):
        idx_flat = np.zeros(total_cap, dtype=np.int16)
        rel_flat = np.full(total_cap, -1.0, dtype=np.float32)
        pos = 0
        for (b, k, cap) in layout:
            g = (c * nb + b) * 2 + k
            s0, n = group_starts[g], group_sizes[g]
            idx_flat[pos:pos + n] = bidx_s[s0:s0 + n].astype(np.int16)
            rel_flat[pos:pos + n] = rel_s[s0:s0 + n].astype(np.float32)
            pos += cap
        assert pos == total_cap

        # wrap indices per call: idx j of a call sits at [j % 16, j // 16]
        idx_w = np.zeros((16, total_cap // 16), dtype=np.int16)
        for (nch, off16, _, _) in calls:
            cc = nch * 128
            seg = idx_flat[off16 * 16: off16 * 16 + cc].reshape(cc // 16, 16).T
            idx_w[:, off16: off16 + cc // 16] = seg
        idx_w = np.tile(idx_w, (8, 1))
        rel_t = rel_flat.reshape(n_chunks, 128).T.astype(BF)  # [128, n_chunks]

        deg = np.zeros(npc, dtype=np.float32)
        m = core == c
        np.add.at(deg, ldst[m], 1.0)
        mask = np.zeros(npc, dtype=np.float32)
        lo = c * npc
        mask[: max(0, min(npc, cfg.n_nodes - lo))] = 1.0
        per_core_arr.append(dict(idxs=idx_w, dstrel=rel_t,
                                 mask=mask[None, :].astype(BF),
                                 deg_bcast=np.tile(deg[None, :], (128, 1)).astype(BF)))

    meta = dict(caps=caps, calls=calls, chunk_block=chunk_block,
                sblocks=sblocks, n_chunks=n_chunks, total_cap=total_cap)
    return meta, per_core_arr


def build_program(cfg: Cfg, meta):
    C, npc, nb = cfg.n_cores, cfg.npc, cfg.n_blocks
    FEAT, HID, L = cfg.feat, cfg.hid, cfg.layers
    n_chunks, total_cap = meta["n_chunks"], meta["total_cap"]
    calls, chunk_block, sblocks = meta["calls"], meta["chunk_block"], meta["sblocks"]
    max_nch = max(c[0] for c in calls)

    # first/last chunk index per block (chunks of one block are contiguous
    # except for the bank split; find global first/last over all its chunks)
    first_chunk = {}
    last_chunk = {}
    for t, b in enumerate(chunk_block):
        first_chunk.setdefault(b, t)
        last_chunk[b] = t

    nc = bacc.Bacc("TRN2", target_bir_lowering=False, debug=False, num_devices=C,
                   dynamic_dma_scratch_size=cfg.dma_scratch,
                   num_swdge_queues=cfg.n_queues)

    def inp(name, shape, dt=F32):
        return nc.dram_tensor(name, shape, dt, kind="ExternalInput").ap()

    at_d = inp("at_own", [FEAT, npc])
    embW_d = inp("emb_W", [FEAT, HID])
    embb_d = inp("emb_b", [1, HID])
    mtop_d = inp("msg_top", [L * HID, HID], BF16)
    mbot_d = inp("msg_bot", [L * HID, HID], BF16)
    mb_d = inp("msg_bias", [L, HID], BF16)
    utop_d = inp("upd_top", [L * HID, HID], BF16)
    ubot_d = inp("upd_bot", [L * HID, HID], BF16)
    ub_d = inp("upd_bias", [L, HID], BF16)
    ow1_d = inp("out_W1", [HID, HID // 2])
    ob1_d = inp("out_b1", [HID // 2, 1])
    ow2_d = inp("out_W2", [HID // 2, HID // 4])
    ob2_d = inp("out_b2", [HID // 4, 1])
    ow3_d = inp("out_W3", [HID // 4, 1])
    ob3_d = inp("out_b3", [1, 1])
    mask_d = inp("mask", [1, npc], BF16)
    degb_d = inp("deg_bcast", [128, npc], BF16)
    idx_d = inp("idxs", [128, total_cap // 16], I16)
    rel_d = inp("dstrel", [128, n_chunks], BF16)
    out_d = nc.dram_tensor("result", [1, 1], F32, kind="ExternalOutput").ap()

    with tile.TileContext(nc) as tc, ExitStack() as ctx:
        const = ctx.enter_context(tc.tile_pool(name="const", bufs=1))
        gpool = ctx.enter_context(tc.tile_pool(name="g", bufs=2))
        ohpool = ctx.enter_context(tc.tile_pool(name="oh", bufs=2))
        spool = ctx.enter_context(tc.tile_pool(name="s", bufs=8))
        rpool = ctx.enter_context(tc.tile_pool(name="rows", bufs=4))
        ps_s = ctx.enter_context(tc.tile_pool(name="ps_s", bufs=6, space="PSUM"))
        ps_d = ctx.enter_context(tc.tile_pool(name="ps_d", bufs=2, space="PSUM"))
        dram = ctx.enter_context(tc.tile_pool(name="dram", bufs=2, space="DRAM"))

        def load_const(name, ap_dram, shape, dt=F32):
            t = const.tile(shape, dt, name=name, tag=name)
            nc.sync.dma_start(out=t[:], in_=ap_dram)
            return t

        embW_sb = load_const("embW_sb", embW_d[:], [FEAT, HID])
        embb_sb = load_const("embb_sb", embb_d[:], [1, HID])
        mtop_sb = [load_const(f"mtop{l}", mtop_d[l * HID:(l + 1) * HID, :], [HID, HID], BF16) for l in range(L)]
        mbot_sb = [load_const(f"mbot{l}", mbot_d[l * HID:(l + 1) * HID, :], [HID, HID], BF16) for l in range(L)]
        mb_sb = [load_const(f"mb{l}", mb_d[l:l + 1, :], [1, HID], BF16) for l in range(L)]
        utop_sb = [load_const(f"utop{l}", utop_d[l * HID:(l + 1) * HID, :], [HID, HID], BF16) for l in range(L)]
        ubot_sb = [load_const(f"ubot{l}", ubot_d[l * HID:(l + 1) * HID, :], [HID, HID], BF16) for l in range(L)]
        ub_sb = [load_const(f"ub{l}", ub_d[l:l + 1, :], [1, HID], BF16) for l in range(L)]
        ow1_sb = load_const("ow1_sb", ow1_d[:], [HID, HID // 2])
        ob1_sb = load_const("ob1_sb", ob1_d[:], [HID // 2, 1])
        ow2_sb = load_const("ow2_sb", ow2_d[:], [HID // 2, HID // 4])
        ob2_sb = load_const("ob2_sb", ob2_d[:], [HID // 4, 1])
        ow3_sb = load_const("ow3_sb", ow3_d[:], [HID // 4, 1])
        ob3_sb = load_const("ob3_sb", ob3_d[:], [1, 1])
        mask_sb = load_const("mask_sb", mask_d[:], [1, npc], BF16)
        degb_sb = load_const("degb_sb", degb_d[:], [128, npc], BF16)
        deg_sb = degb_sb[0:1, :]
        idx_sb = load_const("idx_sb", idx_d[:], [128, total_cap // 16], I16)
        rel_sb = load_const("rel_sb", rel_d[:], [128, n_chunks], BF16)

        iotah_sb = const.tile([128, 128], BF16)
        nc.gpsimd.iota(iotah_sb[:], [[1, 128]], channel_multiplier=0,
                       allow_small_or_imprecise_dtypes=True)
        iota_sb = const.tile([128, 128], F32)
        nc.gpsimd.iota(iota_sb[:], [[1, 128]], channel_multiplier=0,
                       allow_small_or_imprecise_dtypes=True)
        iota_col = const.tile([128, 1], F32)
        nc.gpsimd.iota(iota_col[:], [[1, 1]], channel_multiplier=1,
                       allow_small_or_imprecise_dtypes=True)
        ident_sb = const.tile([128, 128], F32)
        nc.vector.tensor_scalar(ident_sb[:], iota_sb[:], iota_col[:], None,
                                op0=mybir.AluOpType.is_equal)

        hT = const.tile([128, npc], F32)  # h transposed, own nodes

        def alloc_rows():
            return dram.tile([npc, HID], BF16, tag="rows_d", name="rows_dram")

        def emit_block(rows_dram, b):
            """Transpose one updated hT block to row-major and DMA to DRAM."""
            blk = slice(b * 128, (b + 1) * 128)
            pt = ps_d.tile([128, 512], F32, tag="d")
            nc.tensor.transpose(pt[:, :128], hT[:, blk], ident_sb[:])
            r_sb = rpool.tile([128, 128], BF16, tag="r")
            nc.scalar.copy(r_sb[:], pt[:, :128])
            nc.sync.dma_start(out=rows_dram[blk, :], in_=r_sb[:])

        ra, rb = cfg.rows_a, npc - cfg.rows_a

        def allgather_half(rows_dram, k):
            rows = rows_dram[0:ra, :] if k == 0 else rows_dram[ra:npc, :]
            n = ra if k == 0 else rb
            hf = dram.tile([C * n, HID], BF16, tag=f"hfull{k}_d",
                           addr_space="Shared", name="hf")
            nc.gpsimd.collective_compute(
                "AllGather", mybir.AluOpType.bypass,
                replica_groups=[list(range(C))],
                ins=[rows.opt()], outs=[hf.opt()],
            )
            return hf

        # ---- embedding: hT = emb_W^T @ at_own + emb_b (mask-broadcast) ----
        with tc.tile_pool(name="atp", bufs=1) as atp:
            at_sb = atp.tile([FEAT, npc], F32, name="at_sb")
            nc.sync.dma_start(out=at_sb[:], in_=at_d[:])
            maskf_sb = atp.tile([1, npc], F32, name="maskf_sb")
            nc.vector.tensor_copy(maskf_sb[:], mask_sb[:])
            for j0 in range(0, npc, 512):
                jn = min(512, npc - j0)
                pe = ps_d.tile([128, 512], F32, tag="d", name="pe")
                nc.tensor.matmul(pe[:, :jn], embW_sb[:, :], at_sb[:, j0:j0 + jn],
                                 start=True, stop=False)
                nc.tensor.matmul(pe[:, :jn], embb_sb[:, :], maskf_sb[:, j0:j0 + jn],
                                 start=False, stop=True)
                nc.scalar.copy(hT[:, j0:j0 + jn], pe[:, :jn])
        rows0 = alloc_rows()
        hfull = [None, None]
        for b in range(nb):
            emit_block(rows0, b)
            if b == ra // 128 - 1:
                hfull[0] = allgather_half(rows0, 0)
        hfull[1] = allgather_half(rows0, 1)

        # ---- message passing layers ----
        for l in range(L):
            psums = {}
            rows_dram = alloc_rows() if l < L - 1 else None
            next_hfull = [None, None]
            flushed = 0
            for ci, (nch, off16, chunk0, k) in enumerate(calls):
                cc = nch * 128
                g = gpool.tile([128, cfg.gcap // 128, 128], BF16, tag="g")
                nc.gpsimd.dma_gather(
                    g[:, :nch, :],
                    hfull[k][:, :],
                    idx_sb[:, off16: off16 + cc // 16],
                    num_idxs=cc, num_idxs_reg=cc, elem_size=HID,
                    queue_num=ci % cfg.n_queues,
                )
                oh = ohpool.tile([128, cfg.gcap // 128, 128], BF16, tag="oh")
                nc.vector.tensor_tensor(
                    oh[:, :nch, :],
                    iotah_sb[:, :].unsqueeze(1).broadcast_to([128, nch, 128]),
                    rel_sb[:, chunk0:chunk0 + nch].unsqueeze(2).broadcast_to([128, nch, 128]),
                    op=mybir.AluOpType.is_equal)
                for t in range(nch):
                    b = chunk_block[chunk0 + t]
                    if first_chunk[b] == chunk0 + t:
                        psums[b] = ps_s.tile([128, 128], F32, tag="ps_s",
                                             name=f"psum_{b % 16}")
                    nc.tensor.matmul(psums[b][:], g[:, t, :], oh[:, t, :],
                                     start=(first_chunk[b] == chunk0 + t),
                                     stop=(last_chunk[b] == chunk0 + t))
                # when this call closes a super-block's bank-1 run, flush its
                # blocks' dense ops
                next_chunk0 = (calls[ci + 1][2] if ci + 1 < len(calls)
                               else n_chunks)
                done_blocks = [b for b in list(psums)
                               if last_chunk[b] < next_chunk0]
                for b in sorted(done_blocks):
                    blk = slice(b * 128, (b + 1) * 128)
                    s_sb = spool.tile([128, 128], BF16, tag="s")
                    nc.scalar.copy(s_sb[:], psums.pop(b)[:])
                    hTb_sb = spool.tile([128, 128], BF16, tag="s", name="hTb_sb")
                    nc.scalar.copy(hTb_sb[:], hT[:, blk])
                    hdeg_sb = spool.tile([128, 128], BF16, tag="s", name="hdeg_sb")
                    nc.vector.tensor_mul(hdeg_sb[:], hTb_sb[:], degb_sb[:, blk])
                    pa = ps_d.tile([128, 512], F32, tag="d")
                    nc.tensor.matmul(pa[:, :128], mtop_sb[l][:], s_sb[:], start=True, stop=False)
                    nc.tensor.matmul(pa[:, :128], mbot_sb[l][:], hdeg_sb[:], start=False, stop=False)
                    nc.tensor.matmul(pa[:, :128], mb_sb[l][:], deg_sb[:, blk], start=False, stop=True)
                    agg_sb = spool.tile([128, 128], BF16, tag="s")
                    nc.scalar.copy(agg_sb[:], pa[:, :128])

                    pu = ps_d.tile([128, 512], F32, tag="d")
                    nc.tensor.matmul(pu[:, :128], utop_sb[l][:], hTb_sb[:], start=True, stop=False)
                    nc.tensor.matmul(pu[:, :128], ubot_sb[l][:], agg_sb[:], start=False, stop=False)
                    nc.tensor.matmul(pu[:, :128], ub_sb[l][:], mask_sb[:, blk], start=False, stop=True)
                    relu_sb = spool.tile([128, 128], F32, tag="s")
                    nc.scalar.activation(relu_sb[:], pu[:, :128],
                                         mybir.ActivationFunctionType.Relu)
                    nc.vector.tensor_add(hT[:, blk], relu_sb[:], hT[:, blk])
                    if rows_dram is not None:
                        emit_block(rows_dram, b)
                        flushed += 1
                        if flushed == ra // 128:
                            next_hfull[0] = allgather_half(rows_dram, 0)
            assert not psums
            # blocks with zero chunks (no in-edges anywhere): still need dense
            for b in range(nb):
                if b in first_chunk:
                    continue
                blk = slice(b * 128, (b + 1) * 128)
                hTb_sb = spool.tile([128, 128], BF16, tag="s", name="hTb_sb")
                nc.scalar.copy(hTb_sb[:], hT[:, blk])
                pu = ps_d.tile([128, 512], F32, tag="d")
                nc.tensor.matmul(pu[:, :128], utop_sb[l][:], hTb_sb[:], start=True, stop=False)
                nc.tensor.matmul(pu[:, :128], ub_sb[l][:], mask_sb[:, blk], start=False, stop=True)
                relu_sb = spool.tile([128, 128], F32, tag="s")
                nc.scalar.activation(relu_sb[:], pu[:, :128],
                                     mybir.ActivationFunctionType.Relu)
                nc.vector.tensor_add(hT[:, blk], relu_sb[:], hT[:, blk])
                if rows_dram is not None:
                    emit_block(rows_dram, b)
                    flushed += 1
                    if flushed == ra // 128:
                        next_hfull[0] = allgather_half(rows_dram, 0)
            if rows_dram is not None:
                next_hfull[1] = allgather_half(rows_dram, 1)
                assert next_hfull[0] is not None
                hfull = next_hfull

        # ---- readout: g = mean(h) ; out = MLP(g) ----
        part_sb = spool.tile([128, 1], F32, tag="s")
        nc.vector.tensor_reduce(part_sb[:], hT[:, :], axis=mybir.AxisListType.X,
                                op=mybir.AluOpType.add)
        part_dram = dram.tile([128, 1], F32, tag="pt_d")
        gall_dram = dram.tile([C * 128, 1], F32, tag="ga_d", addr_space="Shared")
        nc.sync.dma_start(out=part_dram[:], in_=part_sb[:])
        # AllGather (one phase) + local reduce beats AllReduce (two phases)
        # at this tiny, latency-bound size
        nc.gpsimd.collective_compute(
            "AllGather", mybir.AluOpType.bypass,
            replica_groups=[list(range(C))],
            ins=[part_dram.opt()], outs=[gall_dram.opt()],
        )
        gall_sb = spool.tile([128, C], F32, tag="s", name="gall_sb")
        for c in range(C):
            nc.sync.dma_start(out=gall_sb[:, c:c + 1],
                              in_=gall_dram[c * 128:(c + 1) * 128, :])
        gsum_sb = spool.tile([128, 1], F32, tag="s")
        nc.vector.tensor_reduce(gsum_sb[:], gall_sb[:, :],
                                axis=mybir.AxisListType.X,
                                op=mybir.AluOpType.add)

        p1 = ps_d.tile([128, 512], F32, tag="d")
        nc.tensor.matmul(p1[:HID // 2, :1], ow1_sb[:], gsum_sb[:], start=True, stop=True)
        o1_sb = spool.tile([HID // 2, 1], F32, tag="o1")
        nc.scalar.activation(o1_sb[:], p1[:HID // 2, :1],
                             mybir.ActivationFunctionType.Relu,
                             bias=ob1_sb[:], scale=1.0 / cfg.n_nodes)
        p2 = ps_d.tile([128, 512], F32, tag="d")
        nc.tensor.matmul(p2[:HID // 4, :1], ow2_sb[:], o1_sb[:], start=True, stop=True)
        o2_sb = spool.tile([HID // 4, 1], F32, tag="o2")
        nc.scalar.activation(o2_sb[:], p2[:HID // 4, :1],
                             mybir.ActivationFunctionType.Relu, bias=ob2_sb[:])
        p3 = ps_d.tile([128, 512], F32, tag="d")
        nc.tensor.matmul(p3[:1, :1], ow3_sb[:], o2_sb[:], start=True, stop=True)
        o3_sb = spool.tile([1, 1], F32, tag="o3")
        nc.scalar.activation(o3_sb[:], p3[:1, :1],
                             mybir.ActivationFunctionType.Identity, bias=ob3_sb[:])
        nc.sync.dma_start(out=out_d[:], in_=o3_sb[:])

    nc.compile()
    return nc


def make_in_maps(cfg: Cfg, inputs, per_core_arr):
    C, npc, FEAT, HID, L = cfg.n_cores, cfg.npc, cfg.feat, cfg.hid, cfg.layers
    af = np.asarray(inputs["atom_features"], np.float32)
    at_pad = np.zeros((cfg.n_pad, FEAT), np.float32)
    at_pad[:cfg.n_nodes] = af
    msg_W = np.asarray(inputs["msg_W"], np.float32)
    upd_W = np.asarray(inputs["upd_W"], np.float32)
    shared = dict(
        emb_W=np.asarray(inputs["emb_W"], np.float32),
        emb_b=np.asarray(inputs["emb_b"], np.float32)[None, :],
        msg_top=msg_W[:, :HID, :].reshape(L * HID, HID).astype(BF),
        msg_bot=msg_W[:, HID:, :].reshape(L * HID, HID).astype(BF),
        msg_bias=np.asarray(inputs["msg_b"], np.float32).astype(BF),
        upd_top=upd_W[:, :HID, :].reshape(L * HID, HID).astype(BF),
        upd_bot=upd_W[:, HID:, :].reshape(L * HID, HID).astype(BF),
        upd_bias=np.asarray(inputs["upd_b"], np.float32).astype(BF),
        out_W1=np.asarray(inputs["out_W1"], np.float32),
        out_b1=np.asarray(inputs["out_b1"], np.float32)[:, None],
        out_W2=np.asarray(inputs["out_W2"], np.float32),
        out_b2=np.asarray(inputs["out_b2"], np.float32)[:, None],
        out_W3=np.asarray(inputs["out_W3"], np.float32),
        out_b3=np.asarray(inputs["out_b3"], np.float32)[:, None],
    )
    in_maps = []
    for c in range(C):
        d = dict(shared)
        d["at_own"] = at_pad[c * npc:(c + 1) * npc].T.copy()
        d.update(per_core_arr[c])
        in_maps.append(d)
    return in_maps


_prog_cache = {}


def run(cfg: Cfg, inputs, trace=False):
    meta, per_core_arr = plan_edges(cfg, np.asarray(inputs["edge_indices"]))
    key = (cfg, hashlib.sha1(meta["caps"].tobytes()).hexdigest())
    if key not in _prog_cache:
        _prog_cache[key] = build_program(cfg, meta)
    nc = _prog_cache[key]
    in_maps = make_in_maps(cfg, inputs, per_core_arr)
    res = run_bass_kernel_spmd(nc, in_maps, core_ids=list(range(cfg.n_cores)),
                               trace=trace)
    out = res.results[0]["result"].astype(np.float32)
    return out, res


def kernel(**inputs) -> np.ndarray:
    out, _ = run(Cfg(), inputs)
    return out


# revision 39
# speedup vs baseline: 1.4352x; 1.0047x over previous
"""Trainium2 Bass kernel for EnhancedGNNModel (3-layer GNN message passing).

Strategy (8 NeuronCores, SPMD):
  - Nodes are sharded by dst range: core c owns rows [c*NPC, (c+1)*NPC).
  - Edges are sharded by dst: each core aggregates messages for its own nodes.
  - Per-edge message Linear(concat(h_src, h_dst)) @ W is split algebraically:
        msgs = h[src] @ W_top + h[dst] @ W_bot + b
    so segment_sum(msgs, dst) = (sum_{e->d} h[src]) @ W_top
                                + deg[d] * (h[d] @ W_bot) + deg[d] * b.
    Only S[d] = sum of gathered h[src] rows needs edge-granularity work.
  - Edges are grouped per (super-block of SB dst blocks, src bank); one
    dma_gather call per group pulls h[src] rows into a [128, nch, 128] SBUF
    tile; one bulk is_equal builds all nch one-hot matrices at once; per-chunk
    PE matmuls accumulate S^T per dst block in PSUM.
  - h lives transposed ([HID, node]) in SBUF for all dense matmuls; dense
    weights run in bf16. Updated rows are transposed back via the PE, written
    to DRAM, and AllGathered so every core has the full h for the next
    layer's gathers.
"""
import hashlib
import ml_dtypes
import numpy as np
from contextlib import ExitStack
from dataclasses import dataclass

import concourse.bass as bass
import concourse.tile as tile
from concourse import bacc, mybir
from concourse.bass_utils import run_bass_kernel_spmd

F32 = mybir.dt.float32
BF16 = mybir.dt.bfloat16
I16 = mybir.dt.int16
BF = ml_dtypes.bfloat16


@dataclass(frozen=True)
class Cfg:
    n_nodes: int = 50000
    n_edges: int = 800000
    feat: int = 64
    hid: int = 128
    layers: int = 3
    n_cores: int = 8
    sb: int = 5  # dst blocks per super-block (one gather group per bank)
    gcap: int = 1024  # max indices per dma_gather call (ring-capacity limit)
    dma_scratch: int = 16384
    n_queues: int = 1

    @property
    def npc(self):  # nodes per core (multiple of 128)
        per = -(-self.n_nodes // self.n_cores)
        return -(-per // 128) * 128

    @property
    def n_pad(self):
        return self.npc * self.n_cores

    @property
    def rows_a(self):  # per-core rows in bank A (blocks 0..rows_a/128-1)
        # as large as the int16 gather-index limit allows: AG-A is issued
        # mid-layer (hideable), so the layer-boundary wait is only AG-B
        r = min((32768 // self.n_cores) // 128 * 128, self.npc - 128)
        assert r * self.n_cores <= 32768, "dma_gather int16 index limit"
        assert (self.npc - r) * self.n_cores <= 32768
        return r

    @property
    def n_blocks(self):
        return self.npc // 128


def plan_edges(cfg: Cfg, edge_indices: np.ndarray):
    """Host-side sharding plan. Edge order per core:
    super-block s -> bank k -> block b -> edges (padded to 128 per (b,k)).
    Returns (meta, per_core). meta['calls'] is a list of
    (nch, off16, chunk0, bank) and meta['chunk_block'] maps chunk -> block.
    """
    src = np.asarray(edge_indices[0], dtype=np.int64)
    dst = np.asarray(edge_indices[1], dtype=np.int64)
    C, npc, nb = cfg.n_cores, cfg.npc, cfg.n_blocks
    ra, rb = cfg.rows_a, cfg.npc - cfg.rows_a

    core = dst // npc
    ldst = dst - core * npc
    block = ldst >> 7
    dst_rel = ldst & 127
    core_s = src // npc
    r_s = src - core_s * npc
    bank = (r_s >= ra).astype(np.int64)
    bidx = np.where(bank == 0, core_s * ra + r_s, core_s * rb + (r_s - ra))

    counts = np.zeros((C, nb, 2), dtype=np.int64)
    np.add.at(counts, (core, block, bank), 1)
    caps = counts.max(axis=0)  # [nb, 2]
    caps = np.where(caps > 0, ((caps + 127) // 128) * 128, 0).astype(np.int64)

    key = (core * nb + block) * 2 + bank
    order = np.argsort(key, kind="stable")
    bidx_s, rel_s = bidx[order], dst_rel[order]
    group_sizes = np.bincount(key[order], minlength=C * nb * 2)
    group_starts = np.concatenate([[0], np.cumsum(group_sizes)])

    # super-block partition of blocks
    sblocks = [list(range(s, min(s + cfg.sb, nb)))
               for s in range(0, nb, cfg.sb)]

    # static layout: order of (s, k, b) groups; each group caps[b,k] slots
    layout = []  # (b, k, cap)
    for bs in sblocks:
        for k in range(2):
            for b in bs:
                if caps[b, k]:
                    layout.append((b, k, int(caps[b, k])))
    total_cap = sum(g[2] for g in layout)
    n_chunks = total_cap // 128

    # chunk -> block map and call list (calls split at gcap, never across
    # (s, k) boundaries; all chunks of one call share the bank)
    chunk_block = []
    calls = []  # (nch, off16, chunk0, bank)
    off = 0
    for bs in sblocks:
        for k in range(2):
            groups = [g for g in layout if g[0] in bs and g[1] == k]
            run = sum(g[2] for g in groups)
            if run == 0:
                continue
            for g in groups:
                chunk_block += [g[0]] * (g[2] // 128)
            q = 0
            while q < run:
                cc = min(cfg.gcap, run - q)
                calls.append((cc // 128, (off + q) // 16, (off + q) // 128, k))
                q += cc
            off += run
    assert off == total_cap and len(chunk_block) == n_chunks

    per_core_arr = []
    for c in range(C):
        idx_flat = np.zeros(total_cap, dtype=np.int16)
        rel_flat = np.full(total_cap, -1.0, dtype=np.float32)
        pos = 0
        for (b, k, cap) in layout:
            g = (c * nb + b) * 2 + k
            s0, n = group_starts[g], group_sizes[g]
            idx_flat[pos:pos + n] = bidx_s[s0:s0 + n].astype(np.int16)
            rel_flat[pos:pos + n] = rel_s[s0:s0 + n].astype(np.float32)
            pos += cap
        assert pos == total_cap

        # wrap indices per call: idx j of a call sits at [j % 16, j // 16]
        idx_w = np.zeros((16, total_cap // 16), dtype=np.int16)
        for (nch, off16, _, _) in calls:
            cc = nch * 128
            seg = idx_flat[off16 * 16: off16 * 16 + cc].reshape(cc // 16, 16).T
            idx_w[:, off16: off16 + cc // 16] = seg
        idx_w = np.tile(idx_w, (8, 1))
        rel_t = rel_flat.reshape(n_chunks, 128).T.astype(BF)  # [128, n_chunks]

        deg = np.zeros(npc, dtype=np.float32)
        m = core == c
        np.add.at(deg, ldst[m], 1.0)
        mask = np.zeros(npc, dtype=np.float32)
        lo = c * npc
        mask[: max(0, min(npc, cfg.n_nodes - lo))] = 1.0
        per_core_arr.append(dict(idxs=idx_w, dstrel=rel_t,
                                 mask=mask[None, :].astype(BF),
                                 deg_bcast=np.tile(deg[None, :], (128, 1)).astype(BF)))

    meta = dict(caps=caps, calls=calls, chunk_block=chunk_block,
                sblocks=sblocks, n_chunks=n_chunks, total_cap=total_cap)
    return meta, per_core_arr


def build_program(cfg: Cfg, meta):
    C, npc, nb = cfg.n_cores, cfg.npc, cfg.n_blocks
    FEAT, HID, L = cfg.feat, cfg.hid, cfg.layers
    n_chunks, total_cap = meta["n_chunks"], meta["total_cap"]
    calls, chunk_block, sblocks = meta["calls"], meta["chunk_block"], meta["sblocks"]
    max_nch = max(c[0] for c in calls)

    # first/last chunk index per block (chunks of one block are contiguous
    # except for the bank split; find global first/last over all its chunks)
    first_chunk = {}
    last_chunk = {}
    for t, b in enumerate(chunk_block):
        first_chunk.setdefault(b, t)
        last_chunk[b] = t

    nc = bacc.Bacc("TRN2", target_bir_lowering=False, debug=False, num_devices=C,
                   dynamic_dma_scratch_size=cfg.dma_scratch,
                   num_swdge_queues=cfg.n_queues)

    def inp(name, shape, dt=F32):
        return nc.dram_tensor(name, shape, dt, kind="ExternalInput").ap()

    at_d = inp("at_own", [FEAT, npc])
    embW_d = inp("emb_W", [FEAT, HID])
    embb_d = inp("emb_b", [1, HID])
    mtop_d = inp("msg_top", [L * HID, HID], BF16)
    mbot_d = inp("msg_bot", [L * HID, HID], BF16)
    mb_d = inp("msg_bias", [L, HID], BF16)
    utop_d = inp("upd_top", [L * HID, HID], BF16)
    ubot_d = inp("upd_bot", [L * HID, HID], BF16)
    ub_d = inp("upd_bias", [L, HID], BF16)
    ow1_d = inp("out_W1", [HID, HID // 2])
    ob1_d = inp("out_b1", [HID // 2, 1])
    ow2_d = inp("out_W2", [HID // 2, HID // 4])
    ob2_d = inp("out_b2", [HID // 4, 1])
    ow3_d = inp("out_W3", [HID // 4, 1])
    ob3_d = inp("out_b3", [1, 1])
    mask_d = inp("mask", [1, npc], BF16)
    degb_d = inp("deg_bcast", [128, npc], BF16)
    idx_d = inp("idxs", [128, total_cap // 16], I16)
    rel_d = inp("dstrel", [128, n_chunks], BF16)
    out_d = nc.dram_tensor("result", [1, 1], F32, kind="ExternalOutput").ap()

    with tile.TileContext(nc) as tc, ExitStack() as ctx:
        const = ctx.enter_context(tc.tile_pool(name="const", bufs=1))
        gpool = ctx.enter_context(tc.tile_pool(name="g", bufs=2))
        ohpool = ctx.enter_context(tc.tile_pool(name="oh", bufs=2))
        spool = ctx.enter_context(tc.tile_pool(name="s", bufs=8))
        rpool = ctx.enter_context(tc.tile_pool(name="rows", bufs=4))
        ps_s = ctx.enter_context(tc.tile_pool(name="ps_s", bufs=6, space="PSUM"))
        ps_d = ctx.enter_context(tc.tile_pool(name="ps_d", bufs=2, space="PSUM"))
        dram = ctx.enter_context(tc.tile_pool(name="dram", bufs=2, space="DRAM"))

        def load_const(name, ap_dram, shape, dt=F32):
            t = const.tile(shape, dt, name=name, tag=name)
            nc.sync.dma_start(out=t[:], in_=ap_dram)
            return t

        embW_sb = load_const("embW_sb", embW_d[:], [FEAT, HID])
        embb_sb = load_const("embb_sb", embb_d[:], [1, HID])
        mtop_sb = [load_const(f"mtop{l}", mtop_d[l * HID:(l + 1) * HID, :], [HID, HID], BF16) for l in range(L)]
        mbot_sb = [load_const(f"mbot{l}", mbot_d[l * HID:(l + 1) * HID, :], [HID, HID], BF16) for l in range(L)]
        mb_sb = [load_const(f"mb{l}", mb_d[l:l + 1, :], [1, HID], BF16) for l in range(L)]
        utop_sb = [load_const(f"utop{l}", utop_d[l * HID:(l + 1) * HID, :], [HID, HID], BF16) for l in range(L)]
        ubot_sb = [load_const(f"ubot{l}", ubot_d[l * HID:(l + 1) * HID, :], [HID, HID], BF16) for l in range(L)]
        ub_sb = [load_const(f"ub{l}", ub_d[l:l + 1, :], [1, HID], BF16) for l in range(L)]
        ow1_sb = load_const("ow1_sb", ow1_d[:], [HID, HID // 2])
        ob1_sb = load_const("ob1_sb", ob1_d[:], [HID // 2, 1])
        ow2_sb = load_const("ow2_sb", ow2_d[:], [HID // 2, HID // 4])
        ob2_sb = load_const("ob2_sb", ob2_d[:], [HID // 4, 1])
        ow3_sb = load_const("ow3_sb", ow3_d[:], [HID // 4, 1])
        ob3_sb = load_const("ob3_sb", ob3_d[:], [1, 1])
        mask_sb = load_const("mask_sb", mask_d[:], [1, npc], BF16)
        degb_sb = load_const("degb_sb", degb_d[:], [128, npc], BF16)
        deg_sb = degb_sb[0:1, :]
        idx_sb = load_const("idx_sb", idx_d[:], [128, total_cap // 16], I16)
        rel_sb = load_const("rel_sb", rel_d[:], [128, n_chunks], BF16)

        iotah_sb = const.tile([128, 128], BF16)
        nc.gpsimd.iota(iotah_sb[:], [[1, 128]], channel_multiplier=0,
                       allow_small_or_imprecise_dtypes=True)
        iota_sb = const.tile([128, 128], F32)
        nc.gpsimd.iota(iota_sb[:], [[1, 128]], channel_multiplier=0,
                       allow_small_or_imprecise_dtypes=True)
        iota_col = const.tile([128, 1], F32)
        nc.gpsimd.iota(iota_col[:], [[1, 1]], channel_multiplier=1,
                       allow_small_or_imprecise_dtypes=True)
        ident_sb = const.tile([128, 128], F32)
        nc.vector.tensor_scalar(ident_sb[:], iota_sb[:], iota_col[:], None,
                                op0=mybir.AluOpType.is_equal)

        hT = const.tile([128, npc], F32)  # h transposed, own nodes

        def alloc_rows():
            return dram.tile([npc, HID], BF16, tag="rows_d", name="rows_dram")

        def emit_block(rows_dram, b):
            """Transpose one updated hT block to row-major and DMA to DRAM."""
            blk = slice(b * 128, (b + 1) * 128)
            pt = ps_d.tile([128, 512], F32, tag="d")
            nc.tensor.transpose(pt[:, :128], hT[:, blk], ident_sb[:])
            r_sb = rpool.tile([128, 128], BF16, tag="r")
            nc.scalar.copy(r_sb[:], pt[:, :128])
            nc.sync.dma_start(out=rows_dram[blk, :], in_=r_sb[:])

        ra, rb = cfg.rows_a, npc - cfg.rows_a

        def allgather_half(rows_dram, k):
            rows = rows_dram[0:ra, :] if k == 0 else rows_dram[ra:npc, :]
            n = ra if k == 0 else rb
            hf = dram.tile([C * n, HID], BF16, tag=f"hfull{k}_d",
                           addr_space="Shared", name="hf")
            nc.gpsimd.collective_compute(
                "AllGather", mybir.AluOpType.bypass,
                replica_groups=[list(range(C))],
                ins=[rows.opt()], outs=[hf.opt()],
            )
            return hf

        # ---- embedding: hT = emb_W^T @ at_own + emb_b (mask-broadcast) ----
        with tc.tile_pool(name="atp", bufs=1) as atp:
            at_sb = atp.tile([FEAT, npc], F32, name="at_sb")
            nc.sync.dma_start(out=at_sb[:], in_=at_d[:])
            maskf_sb = atp.tile([1, npc], F32, name="maskf_sb")
            nc.vector.tensor_copy(maskf_sb[:], mask_sb[:])
            rows0 = alloc_rows()
            hfull = [None, None]
            for j0 in range(0, npc, 512):
                jn = min(512, npc - j0)
                pe = ps_d.tile([128, 512], F32, tag="d", name="pe")
                nc.tensor.matmul(pe[:, :jn], embW_sb[:, :], at_sb[:, j0:j0 + jn],
                                 start=True, stop=False)
                nc.tensor.matmul(pe[:, :jn], embb_sb[:, :], maskf_sb[:, j0:j0 + jn],
                                 start=False, stop=True)
                nc.scalar.copy(hT[:, j0:j0 + jn], pe[:, :jn])
                for b in range(j0 // 128, (j0 + jn) // 128):
                    emit_block(rows0, b)
                    if b == ra // 128 - 1:
                        hfull[0] = allgather_half(rows0, 0)
        hfull[1] = allgather_half(rows0, 1)

        # ---- message passing layers ----
        for l in range(L):
            psums = {}
            rows_dram = alloc_rows() if l < L - 1 else None
            next_hfull = [None, None]
            flushed = 0
            for ci, (nch, off16, chunk0, k) in enumerate(calls):
                cc = nch * 128
                g = gpool.tile([128, cfg.gcap // 128, 128], BF16, tag="g")
                nc.gpsimd.dma_gather(
                    g[:, :nch, :],
                    hfull[k][:, :],
                    idx_sb[:, off16: off16 + cc // 16],
                    num_idxs=cc, num_idxs_reg=cc, elem_size=HID,
                    queue_num=ci % cfg.n_queues,
                )
                oh = ohpool.tile([128, cfg.gcap // 128, 128], BF16, tag="oh")
                nc.vector.tensor_tensor(
                    oh[:, :nch, :],
                    iotah_sb[:, :].unsqueeze(1).broadcast_to([128, nch, 128]),
                    rel_sb[:, chunk0:chunk0 + nch].unsqueeze(2).broadcast_to([128, nch, 128]),
                    op=mybir.AluOpType.is_equal)
                for t in range(nch):
                    b = chunk_block[chunk0 + t]
                    if first_chunk[b] == chunk0 + t:
                        psums[b] = ps_s.tile([128, 128], F32, tag="ps_s",
                                             name=f"psum_{b % 16}")
                    nc.tensor.matmul(psums[b][:], g[:, t, :], oh[:, t, :],
                                     start=(first_chunk[b] == chunk0 + t),
                                     stop=(last_chunk[b] == chunk0 + t))
                # when this call closes a super-block's bank-1 run, flush its
                # blocks' dense ops
                next_chunk0 = (calls[ci + 1][2] if ci + 1 < len(calls)
                               else n_chunks)
                done_blocks = [b for b in list(psums)
                               if last_chunk[b] < next_chunk0]
                for b in sorted(done_blocks):
                    blk = slice(b * 128, (b + 1) * 128)
                    s_sb = spool.tile([128, 128], BF16, tag="s")
                    nc.scalar.copy(s_sb[:], psums.pop(b)[:])
                    hTb_sb = spool.tile([128, 128], BF16, tag="s", name="hTb_sb")
                    nc.scalar.copy(hTb_sb[:], hT[:, blk])
                    hdeg_sb = spool.tile([128, 128], BF16, tag="s", name="hdeg_sb")
                    nc.vector.tensor_mul(hdeg_sb[:], hTb_sb[:], degb_sb[:, blk])
                    pa = ps_d.tile([128, 512], F32, tag="d")
                    nc.tensor.matmul(pa[:, :128], mtop_sb[l][:], s_sb[:], start=True, stop=False)
                    nc.tensor.matmul(pa[:, :128], mbot_sb[l][:], hdeg_sb[:], start=False, stop=False)
                    nc.tensor.matmul(pa[:, :128], mb_sb[l][:], deg_sb[:, blk], start=False, stop=True)
                    agg_sb = spool.tile([128, 128], BF16, tag="s")
                    nc.scalar.copy(agg_sb[:], pa[:, :128])

                    pu = ps_d.tile([128, 512], F32, tag="d")
                    nc.tensor.matmul(pu[:, :128], utop_sb[l][:], hTb_sb[:], start=True, stop=False)
                    nc.tensor.matmul(pu[:, :128], ubot_sb[l][:], agg_sb[:], start=False, stop=False)
                    nc.tensor.matmul(pu[:, :128], ub_sb[l][:], mask_sb[:, blk], start=False, stop=True)
                    relu_sb = spool.tile([128, 128], F32, tag="s")
                    nc.scalar.activation(relu_sb[:], pu[:, :128],
                                         mybir.ActivationFunctionType.Relu)
                    nc.vector.tensor_add(hT[:, blk], relu_sb[:], hT[:, blk])
                    if rows_dram is not None:
                        emit_block(rows_dram, b)
                        flushed += 1
                        if flushed == ra // 128:
                            next_hfull[0] = allgather_half(rows_dram, 0)
            assert not psums
            # blocks with zero chunks (no in-edges anywhere): still need dense
            for b in range(nb):
                if b in first_chunk:
                    continue
                blk = slice(b * 128, (b + 1) * 128)
                hTb_sb = spool.tile([128, 128], BF16, tag="s", name="hTb_sb")
                nc.scalar.copy(hTb_sb[:], hT[:, blk])
                pu = ps_d.tile([128, 512], F32, tag="d")
                nc.tensor.matmul(pu[:, :128], utop_sb[l][:], hTb_sb[:], start=True, stop=False)
                nc.tensor.matmul(pu[:, :128], ub_sb[l][:], mask_sb[:, blk], start=False, stop=True)
                relu_sb = spool.tile([128, 128], F32, tag="s")
                nc.scalar.activation(relu_sb[:], pu[:, :128],
                                     mybir.ActivationFunctionType.Relu)
                nc.vector.tensor_add(hT[:, blk], relu_sb[:], hT[:, blk])
                if rows_dram is not None:
                    emit_block(rows_dram, b)
                    flushed += 1
                    if flushed == ra // 128:
                        next_hfull[0] = allgather_half(rows_dram, 0)
            if rows_dram is not None:
                next_hfull[1] = allgather_half(rows_dram, 1)
                assert next_hfull[0] is not None
                hfull = next_hfull

        # ---- readout: g = mean(h) ; out = MLP(g) ----
        part_sb = spool.tile([128, 1], F32, tag="s")
        nc.vector.tensor_reduce(part_sb[:], hT[:, :], axis=mybir.AxisListType.X,
                                op=mybir.AluOpType.add)
        part_dram = dram.tile([128, 1], F32, tag="pt_d")
        gall_dram = dram.tile([C * 128, 1], F32, tag="ga_d", addr_space="Shared")
        nc.sync.dma_start(out=part_dram[:], in_=part_sb[:])
        # AllGather (one phase) + local reduce beats AllReduce (two phases)
        # at this tiny, latency-bound size
        nc.gpsimd.collective_compute(
            "AllGather", mybir.AluOpType.bypass,
            replica_groups=[list(range(C))],
            ins=[part_dram.opt()], outs=[gall_dram.opt()],
        )
        gall_sb = spool.tile([128, C], F32, tag="s", name="gall_sb")
        for c in range(C):
            nc.sync.dma_start(out=gall_sb[:, c:c + 1],
                              in_=gall_dram[c * 128:(c + 1) * 128, :])
        gsum_sb = spool.tile([128, 1], F32, tag="s")
        nc.vector.tensor_reduce(gsum_sb[:], gall_sb[:, :],
                                axis=mybir.AxisListType.X,
                                op=mybir.AluOpType.add)

        p1 = ps_d.tile([128, 512], F32, tag="d")
        nc.tensor.matmul(p1[:HID // 2, :1], ow1_sb[:], gsum_sb[:], start=True, stop=True)
        o1_sb = spool.tile([HID // 2, 1], F32, tag="o1")
        nc.scalar.activation(o1_sb[:], p1[:HID // 2, :1],
                             mybir.ActivationFunctionType.Relu,
                             bias=ob1_sb[:], scale=1.0 / cfg.n_nodes)
        p2 = ps_d.tile([128, 512], F32, tag="d")
        nc.tensor.matmul(p2[:HID // 4, :1], ow2_sb[:], o1_sb[:], start=True, stop=True)
        o2_sb = spool.tile([HID // 4, 1], F32, tag="o2")
        nc.scalar.activation(o2_sb[:], p2[:HID // 4, :1],
                             mybir.ActivationFunctionType.Relu, bias=ob2_sb[:])
        p3 = ps_d.tile([128, 512], F32, tag="d")
        nc.tensor.matmul(p3[:1, :1], ow3_sb[:], o2_sb[:], start=True, stop=True)
        o3_sb = spool.tile([1, 1], F32, tag="o3")
        nc.scalar.activation(o3_sb[:], p3[:1, :1],
                             mybir.ActivationFunctionType.Identity, bias=ob3_sb[:])
        nc.sync.dma_start(out=out_d[:], in_=o3_sb[:])

    nc.compile()
    return nc


def make_in_maps(cfg: Cfg, inputs, per_core_arr):
    C, npc, FEAT, HID, L = cfg.n_cores, cfg.npc, cfg.feat, cfg.hid, cfg.layers
    af = np.asarray(inputs["atom_features"], np.float32)
    at_pad = np.zeros((cfg.n_pad, FEAT), np.float32)
    at_pad[:cfg.n_nodes] = af
    msg_W = np.asarray(inputs["msg_W"], np.float32)
    upd_W = np.asarray(inputs["upd_W"], np.float32)
    shared = dict(
        emb_W=np.asarray(inputs["emb_W"], np.float32),
        emb_b=np.asarray(inputs["emb_b"], np.float32)[None, :],
        msg_top=msg_W[:, :HID, :].reshape(L * HID, HID).astype(BF),
        msg_bot=msg_W[:, HID:, :].reshape(L * HID, HID).astype(BF),
        msg_bias=np.asarray(inputs["msg_b"], np.float32).astype(BF),
        upd_top=upd_W[:, :HID, :].reshape(L * HID, HID).astype(BF),
        upd_bot=upd_W[:, HID:, :].reshape(L * HID, HID).astype(BF),
        upd_bias=np.asarray(inputs["upd_b"], np.float32).astype(BF),
        out_W1=np.asarray(inputs["out_W1"], np.float32),
        out_b1=np.asarray(inputs["out_b1"], np.float32)[:, None],
        out_W2=np.asarray(inputs["out_W2"], np.float32),
        out_b2=np.asarray(inputs["out_b2"], np.float32)[:, None],
        out_W3=np.asarray(inputs["out_W3"], np.float32),
        out_b3=np.asarray(inputs["out_b3"], np.float32)[:, None],
    )
    in_maps = []
    for c in range(C):
        d = dict(shared)
        d["at_own"] = at_pad[c * npc:(c + 1) * npc].T.copy()
        d.update(per_core_arr[c])
        in_maps.append(d)
    return in_maps


_prog_cache = {}


def run(cfg: Cfg, inputs, trace=False):
    meta, per_core_arr = plan_edges(cfg, np.asarray(inputs["edge_indices"]))
    key = (cfg, hashlib.sha1(meta["caps"].tobytes()).hexdigest())
    if key not in _prog_cache:
        _prog_cache[key] = build_program(cfg, meta)
    nc = _prog_cache[key]
    in_maps = make_in_maps(cfg, inputs, per_core_arr)
    res = run_bass_kernel_spmd(nc, in_maps, core_ids=list(range(cfg.n_cores)),
                               trace=trace)
    out = res.results[0]["result"].astype(np.float32)
    return out, res


def kernel(**inputs) -> np.ndarray:
    out, _ = run(Cfg(), inputs)
    return out
